# revision 25
# baseline (speedup 1.0000x reference)
"""Varlen causal GQA attention on 8 TRN2 NeuronCores.

Problem: 32 q heads, 8 kv heads, head_dim 128, ragged batch (cu_seqlens),
f32. Sharded by KV-head group: core c owns kv head c and q heads
4c..4c+3 — fully data-independent across cores, no collectives.

Per core, blockwise causal attention with all 4 q heads fused via 3D
access patterns (q stored head-interleaved [d, h, t]):
    S^T[h][k, q] = (K_j)^T.T @ Q^T[h]   2 matmuls (head pairs), shared K_j
    P = exp(S * scale)                  ONE ScalarE op over all 4 heads
    causal mask: 0/1 multiply on GpSimd (off the ACT/DVE critical path)
    O^T[h] += V_j @ P[h]                2 matmuls (head pairs), shared V_j
    sums[h] += ones.T @ P[h]            2 M=1 matmuls (head pairs)
Each PSUM bank carries exactly one accumulation chain (head pairs share
a bank through a single 3D-AP matmul). Host does all transposes
(Q^T/K^T in, O^T -> O out), bf16 conversion, and the softmax division.
q groups are 256 wide so PSUM fits: S4 (2 banks x2) + O pairs (2) +
sums pairs (2) = 8 banks.
"""

import math
import os
import sys

sys.path.insert(0, "/opt/trn_rl_repo")

import ml_dtypes
import numpy as np

NUM_HEADS = 32
NUM_KV_HEADS = 8
HEAD_DIM = 128
HEADS_PER_CORE = NUM_HEADS // NUM_KV_HEADS  # 4
N_CORES = 8
BLK = 128
GROUP = 256
SCALE = 1.0 / math.sqrt(HEAD_DIM)

_GRAPH_CACHE = {}


def _build_graph(seq_blocks):
    """Build the SPMD Bacc graph for padded per-seq block counts."""
    from concourse import bacc
    import concourse.mybir as mybir
    from concourse.tile import TileContext

    f32 = mybir.dt.float32
    bf16 = mybir.dt.bfloat16
    T = sum(seq_blocks) * BLK
    n_blocks_total = T // BLK
    H = HEADS_PER_CORE

    nc = bacc.Bacc("TRN2", target_bir_lowering=False, debug=False,
                   num_devices=N_CORES)

    qT_ext = nc.declare_dram_parameter("qT", [BLK, H, T], bf16, isOutput=False)
    kT_ext = nc.declare_dram_parameter("kT", [BLK, T], bf16, isOutput=False)
    v_ext = nc.declare_dram_parameter("v", [T, HEAD_DIM], bf16, isOutput=False)
    mask_ext = nc.declare_dram_parameter("mask", [BLK, H, BLK], bf16,
                                         isOutput=False)
    oT_ext = [
        nc.declare_dram_parameter(f"oT{h}", [BLK, T], f32, isOutput=True)
        for h in range(H)
    ]
    sums_ext = [
        nc.declare_dram_parameter(f"sums{h}", [1, T], f32, isOutput=True)
        for h in range(H)
    ]

    with TileContext(nc) as tc:
        with (
            tc.tile_pool(name="persist", bufs=1) as persist,
            tc.tile_pool(name="p", bufs=4) as p_pool,
            tc.tile_pool(name="ps_s4", bufs=2, space="PSUM") as ps_s4,
            tc.tile_pool(name="ps_o", bufs=1, space="PSUM") as ps_o,
            tc.tile_pool(name="ps_sum", bufs=1, space="PSUM") as ps_sum,
        ):
            kT_sb = persist.tile([BLK, T], bf16)
            v_sb = persist.tile([BLK, n_blocks_total, HEAD_DIM], bf16)
            mask_sb = persist.tile([BLK, H, BLK], bf16)
            qT_sb = persist.tile([BLK, H, T], bf16)
            v_re = v_ext[:].rearrange("(j p) d -> p j d", p=BLK)
            nb0 = seq_blocks[0]
            # seq-0 k + q land first so compute starts early
            nc.sync.dma_start(kT_sb[:, : nb0 * BLK], kT_ext[:, : nb0 * BLK])
            nc.sync.dma_start(
                qT_sb[:, :, : nb0 * BLK], qT_ext[:, :, : nb0 * BLK]
            )
            nc.sync.dma_start(v_sb[:, :nb0, :], v_re[:, :nb0, :])
            nc.scalar.dma_start(mask_sb[:], mask_ext[:])
            if nb0 < n_blocks_total:
                c0 = nb0 * BLK
                nc.scalar.dma_start(kT_sb[:, c0:], kT_ext[:, c0:])
                nc.scalar.dma_start(v_sb[:, nb0:, :], v_re[:, nb0:, :])
                nc.sync.dma_start(qT_sb[:, :, c0:], qT_ext[:, :, c0:])

            ones_f = persist.tile([BLK, 1], f32)
            nc.vector.memset(ones_f[:], 1.0)
            ones_b = persist.tile([BLK, 1], bf16)
            nc.vector.tensor_copy(ones_b[:], ones_f[:])

            ot_stage = [
                persist.tile([BLK, T], f32, name=f"ot_stage{h}")
                for h in range(H)
            ]
            sums_stage = [
                persist.tile([1, T], f32, name=f"sums_stage{h}")
                for h in range(H)
            ]

            seq_off = 0
            for nblk in seq_blocks:
                Ls = nblk * BLK
                for g in range((Ls + GROUP - 1) // GROUP):
                    Q0 = g * GROUP
                    W = min(GROUP, Ls - Q0)
                    jmax = (Q0 + W) // BLK - 1
                    # one PSUM bank per head pair, one acc chain per bank
                    oT_ps = [
                        ps_o.tile([BLK, 2, GROUP], f32, tag=f"ot_ps{pr}",
                                  name="oT_ps")
                        for pr in range(2)
                    ]
                    sums_ps = [
                        ps_sum.tile([1, 2, GROUP], f32, tag=f"sums_ps{pr}",
                                    name="sums_ps")
                        for pr in range(2)
                    ]
                    for j in range(jmax + 1):
                        cs = max(0, BLK * j - Q0)
                        N = W - cs
                        kj = kT_sb[:, seq_off + j * BLK : seq_off + (j + 1) * BLK]
                        vj = v_sb[:, seq_off // BLK + j, :]
                        s4 = ps_s4.tile([BLK, H, GROUP], f32, tag="s4",
                                        name="s4")
                        for pr in range(2):
                            nc.tensor.matmul(
                                s4[:, 2 * pr : 2 * pr + 2, cs : cs + N],
                                kj,
                                qT_sb[
                                    :,
                                    2 * pr : 2 * pr + 2,
                                    seq_off + Q0 + cs : seq_off + Q0 + cs + N,
                                ],
                                start=True,
                                stop=True,
                            )
                        p4 = p_pool.tile([BLK, H, GROUP], bf16, tag="p4",
                                         name="p4")
                        nc.scalar.activation(
                            p4[:, :, :N],
                            s4[:, :, cs : cs + N],
                            mybir.ActivationFunctionType.Exp,
                            scale=SCALE,
                        )
                        if BLK * j >= Q0:  # diagonal: zero upper triangle
                            nc.gpsimd.tensor_mul(
                                p4[:, :, :BLK], p4[:, :, :BLK], mask_sb[:]
                            )
                        last = j == jmax
                        for pr in range(2):
                            nc.tensor.matmul(
                                oT_ps[pr][:, :, cs : cs + N],
                                vj,
                                p4[:, 2 * pr : 2 * pr + 2, :N],
                                start=(j == 0),
                                stop=last,
                            )
                        for pr in range(2):
                            nc.tensor.matmul(
                                sums_ps[pr][:, :, cs : cs + N],
                                ones_b[:],
                                p4[:, 2 * pr : 2 * pr + 2, :N],
                                start=(j == 0),
                                stop=last,
                            )
                    for h in range(H):
                        nc.vector.tensor_copy(
                            ot_stage[h][:, seq_off + Q0 : seq_off + Q0 + W],
                            oT_ps[h // 2][:, h % 2, :W],
                        )
                        nc.vector.tensor_copy(
                            sums_stage[h][:, seq_off + Q0 : seq_off + Q0 + W],
                            sums_ps[h // 2][:, h % 2, :W],
                        )
                # stream this sequence's outputs while later seqs compute
                for h in range(H):
                    nc.sync.dma_start(
                        oT_ext[h][:, seq_off : seq_off + Ls],
                        ot_stage[h][:, seq_off : seq_off + Ls],
                    )
                seq_off += Ls

            for h in range(H):
                nc.sync.dma_start(sums_ext[h][:], sums_stage[h][:])

    nc.finalize()
    return nc


def _install_ntff_hook():
    """Shim antenv.axon_hooks (absent in this container) so trace=True can
    reach the terminal's NRT profiler via libaxon_pjrt.so ctypes."""
    import types

    if "antenv.axon_hooks" in sys.modules:
        return
    import antenv
    from concourse import bass_utils

    mod = types.ModuleType("antenv.axon_hooks")
    state = {"hook": None}
    mod.set_axon_ntff_profile_hook = lambda h: state.__setitem__("hook", h)
    mod.get_axon_ntff_profile_hook = lambda: state["hook"]
    sys.modules["antenv.axon_hooks"] = mod
    antenv.axon_hooks = mod
    bass_utils.upload_artifacts = lambda tmpdir: tmpdir  # zero-egress container
    try:
        if "/root/.axon_site" not in sys.path:
            sys.path.insert(0, "/root/.axon_site")
        from trn_agent_boot.trn_boot import _ntff_profile_via_ctypes

        mod.set_axon_ntff_profile_hook(
            _ntff_profile_via_ctypes("/opt/axon/libaxon_pjrt.so")
        )
    except Exception:
        pass


def kernel(q, k, v, cu_seqlens, max_seqlen):
    from concourse import bass_utils

    q = np.asarray(q, dtype=np.float32)
    k = np.asarray(k, dtype=np.float32)
    v = np.asarray(v, dtype=np.float32)
    cu = np.asarray(cu_seqlens, dtype=np.int64)
    T_host = q.shape[0]
    lengths = np.diff(cu).astype(np.int64)
    all_nblocks = [int((L + BLK - 1) // BLK) for L in lengths]
    T_pad = sum(all_nblocks) * BLK

    # process sequences longest-first: big seq warms the pipeline while the
    # rest of the data streams in, and the tail drains a small seq
    order = sorted(range(len(lengths)), key=lambda s: -all_nblocks[s])
    nblocks = [all_nblocks[s] for s in order]

    # host -> padded device token index map (valid tokens only)
    dev_idx = np.zeros(T_host, dtype=np.int64)
    pad_off = 0
    for s in order:
        L = int(lengths[s])
        dev_idx[cu[s] : cu[s] + L] = pad_off + np.arange(L)
        pad_off += all_nblocks[s] * BLK

    bf16 = ml_dtypes.bfloat16
    qp = np.zeros((T_pad, NUM_HEADS * HEAD_DIM), bf16)
    kp = np.zeros((T_pad, NUM_KV_HEADS * HEAD_DIM), bf16)
    vp = np.zeros((T_pad, NUM_KV_HEADS * HEAD_DIM), bf16)
    qp[dev_idx] = q.astype(bf16)
    kp[dev_idx] = k.astype(bf16)
    vp[dev_idx] = v.astype(bf16)

    mask1 = np.where(
        np.arange(BLK)[:, None] <= np.arange(BLK)[None, :], 1.0, 0.0
    ).astype(bf16)
    mask = np.broadcast_to(
        mask1[:, None, :], (BLK, HEADS_PER_CORE, BLK)
    ).copy()

    key = tuple(nblocks)
    if key not in _GRAPH_CACHE:
        _GRAPH_CACHE[key] = _build_graph(key)
    nc = _GRAPH_CACHE[key]

    in_maps = []
    for c in range(N_CORES):
        m = {"mask": mask}
        m["kT"] = np.ascontiguousarray(kp[:, c * HEAD_DIM : (c + 1) * HEAD_DIM].T)
        m["v"] = np.ascontiguousarray(vp[:, c * HEAD_DIM : (c + 1) * HEAD_DIM])
        # [d, h, t] head-interleaved Q^T so head pairs are 3D-AP adjacent
        qc = qp[:, c * HEADS_PER_CORE * HEAD_DIM : (c + 1) * HEADS_PER_CORE * HEAD_DIM]
        m["qT"] = np.ascontiguousarray(
            qc.reshape(T_pad, HEADS_PER_CORE, HEAD_DIM).transpose(2, 1, 0)
        )
        in_maps.append(m)

    trace = bool(os.environ.get("BASS_TRACE"))
    if trace:
        _install_ntff_hook()
    res = bass_utils.run_bass_kernel_spmd(
        nc, in_maps, core_ids=list(range(N_CORES)), trace=trace
    )
    if trace and res.exec_time_ns is not None:
        print(f"HW exec time: {res.exec_time_ns} ns")
        if res.instructions_and_trace is not None:
            print(f"trace: {res.instructions_and_trace[1]}")

    out = np.empty((T_host, NUM_HEADS * HEAD_DIM), np.float32)
    for c in range(N_CORES):
        r = res.results[c]
        for h in range(HEADS_PER_CORE):
            gh = c * HEADS_PER_CORE + h
            oT = r[f"oT{h}"]  # [128, T_pad] unnormalized
            sums = r[f"sums{h}"][0]  # [T_pad]
            o = (oT[:, dev_idx] / sums[dev_idx][None, :]).T  # [T_host, 128]
            out[:, gh * HEAD_DIM : (gh + 1) * HEAD_DIM] = o
    return out


# revision 30
# speedup vs baseline: 1.0541x; 1.0541x over previous
"""Varlen causal GQA attention on 8 TRN2 NeuronCores.

Problem: 32 q heads, 8 kv heads, head_dim 128, ragged batch (cu_seqlens),
f32. Sharded by KV-head group: core c owns kv head c and q heads
4c..4c+3 — fully data-independent across cores, no collectives.

Per core, blockwise causal attention with all 4 q heads fused via 3D
access patterns (q stored head-interleaved [d, h, t]):
    S^T[h][k, q] = (K_j)^T.T @ Q^T[h]   2 matmuls (head pairs), shared K_j
    P = exp(S * scale)                  ONE ScalarE op over all 4 heads
    causal mask: 0/1 multiply on GpSimd (off the ACT/DVE critical path)
    O^T[h] += V_j @ P[h]                2 matmuls (head pairs), shared V_j
    sums[h] += ones.T @ P[h]            2 M=1 matmuls (head pairs)
Each PSUM bank carries exactly one accumulation chain (head pairs share
a bank through a single 3D-AP matmul). Host does all transposes
(Q^T/K^T in, O^T -> O out), bf16 conversion, and the softmax division.
q groups are 256 wide so PSUM fits: S4 (2 banks x2) + O pairs (2) +
sums pairs (2) = 8 banks.
"""

import math
import os
import sys

sys.path.insert(0, "/opt/trn_rl_repo")

import ml_dtypes
import numpy as np

NUM_HEADS = 32
NUM_KV_HEADS = 8
HEAD_DIM = 128
HEADS_PER_CORE = NUM_HEADS // NUM_KV_HEADS  # 4
N_CORES = 8
BLK = 128
GROUP = 256
SCALE = 1.0 / math.sqrt(HEAD_DIM)

_GRAPH_CACHE = {}


def _build_graph(seq_blocks):
    """Build the SPMD Bacc graph for padded per-seq block counts."""
    from concourse import bacc
    import concourse.mybir as mybir
    from concourse.tile import TileContext

    f32 = mybir.dt.float32
    bf16 = mybir.dt.bfloat16
    T = sum(seq_blocks) * BLK
    n_blocks_total = T // BLK
    H = HEADS_PER_CORE

    nc = bacc.Bacc("TRN2", target_bir_lowering=False, debug=False,
                   num_devices=N_CORES)

    qT_ext = nc.declare_dram_parameter("qT", [BLK, H, T], bf16, isOutput=False)
    kT_ext = nc.declare_dram_parameter("kT", [BLK, T], bf16, isOutput=False)
    v_ext = nc.declare_dram_parameter("v", [T, HEAD_DIM], bf16, isOutput=False)
    mask_ext = nc.declare_dram_parameter("mask", [BLK, H, BLK], bf16,
                                         isOutput=False)
    oT_ext = [
        nc.declare_dram_parameter(f"oT{pr}", [BLK, 2, T], f32, isOutput=True)
        for pr in range(2)
    ]
    sums_ext = [
        nc.declare_dram_parameter(f"sums{pr}", [1, 2, T], f32, isOutput=True)
        for pr in range(2)
    ]

    with TileContext(nc) as tc:
        with (
            tc.tile_pool(name="persist", bufs=1) as persist,
            tc.tile_pool(name="p", bufs=4) as p_pool,
            tc.tile_pool(name="ps_s4", bufs=2, space="PSUM") as ps_s4,
            tc.tile_pool(name="ps_o", bufs=1, space="PSUM") as ps_o,
            tc.tile_pool(name="ps_sum", bufs=1, space="PSUM") as ps_sum,
        ):
            kT_sb = persist.tile([BLK, T], bf16)
            v_sb = persist.tile([BLK, n_blocks_total, HEAD_DIM], bf16)
            mask_sb = persist.tile([BLK, H, BLK], bf16)
            qT_sb = persist.tile([BLK, H, T], bf16)
            v_re = v_ext[:].rearrange("(j p) d -> p j d", p=BLK)
            nb0 = seq_blocks[0]
            # seq-0 k + q land first so compute starts early
            nc.sync.dma_start(kT_sb[:, : nb0 * BLK], kT_ext[:, : nb0 * BLK])
            nc.sync.dma_start(
                qT_sb[:, :, : nb0 * BLK], qT_ext[:, :, : nb0 * BLK]
            )
            nc.sync.dma_start(v_sb[:, :nb0, :], v_re[:, :nb0, :])
            nc.scalar.dma_start(mask_sb[:], mask_ext[:])
            if nb0 < n_blocks_total:
                c0 = nb0 * BLK
                nc.scalar.dma_start(kT_sb[:, c0:], kT_ext[:, c0:])
                nc.scalar.dma_start(v_sb[:, nb0:, :], v_re[:, nb0:, :])
                nc.sync.dma_start(qT_sb[:, :, c0:], qT_ext[:, :, c0:])

            ones_f = persist.tile([BLK, 1], f32)
            nc.vector.memset(ones_f[:], 1.0)
            ones_b = persist.tile([BLK, 1], bf16)
            nc.vector.tensor_copy(ones_b[:], ones_f[:])

            ot_stage = [
                persist.tile([BLK, 2, T], f32, name=f"ot_stage{pr}")
                for pr in range(2)
            ]
            sums_stage = [
                persist.tile([1, 2, T], f32, name=f"sums_stage{pr}")
                for pr in range(2)
            ]

            seq_off = 0
            for nblk in seq_blocks:
                Ls = nblk * BLK
                for g in range((Ls + GROUP - 1) // GROUP):
                    Q0 = g * GROUP
                    W = min(GROUP, Ls - Q0)
                    jmax = (Q0 + W) // BLK - 1
                    # one PSUM bank per head pair, one acc chain per bank
                    oT_ps = [
                        ps_o.tile([BLK, 2, GROUP], f32, tag=f"ot_ps{pr}",
                                  name="oT_ps")
                        for pr in range(2)
                    ]
                    sums_ps = [
                        ps_sum.tile([1, 2, GROUP], f32, tag=f"sums_ps{pr}",
                                    name="sums_ps")
                        for pr in range(2)
                    ]
                    # j descending: diagonal (masked) blocks issue first so
                    # their GpSimd mask latency hides under the full blocks
                    j_first = jmax
                    for j in range(jmax, -1, -1):
                        cs = max(0, BLK * j - Q0)
                        N = W - cs
                        kj = kT_sb[:, seq_off + j * BLK : seq_off + (j + 1) * BLK]
                        vj = v_sb[:, seq_off // BLK + j, :]
                        s4 = ps_s4.tile([BLK, H, GROUP], f32, tag="s4",
                                        name="s4")
                        for pr in range(2):
                            nc.tensor.matmul(
                                s4[:, 2 * pr : 2 * pr + 2, cs : cs + N],
                                kj,
                                qT_sb[
                                    :,
                                    2 * pr : 2 * pr + 2,
                                    seq_off + Q0 + cs : seq_off + Q0 + cs + N,
                                ],
                                start=True,
                                stop=True,
                            )
                        p4 = p_pool.tile([BLK, H, GROUP], bf16, tag="p4",
                                         name="p4")
                        nc.scalar.activation(
                            p4[:, :, :N],
                            s4[:, :, cs : cs + N],
                            mybir.ActivationFunctionType.Exp,
                            scale=SCALE,
                        )
                        if BLK * j >= Q0:  # diagonal: zero upper triangle
                            nc.gpsimd.tensor_mul(
                                p4[:, :, :BLK], p4[:, :, :BLK], mask_sb[:]
                            )
                        for pr in range(2):
                            nc.tensor.matmul(
                                oT_ps[pr][:, :, cs : cs + N],
                                vj,
                                p4[:, 2 * pr : 2 * pr + 2, :N],
                                start=(j == j_first),
                                stop=(j == 0),
                            )
                        for pr in range(2):
                            nc.tensor.matmul(
                                sums_ps[pr][:, :, cs : cs + N],
                                ones_b[:],
                                p4[:, 2 * pr : 2 * pr + 2, :N],
                                start=(j == j_first),
                                stop=(j == 0),
                            )
                    for pr in range(2):
                        nc.vector.tensor_copy(
                            ot_stage[pr][:, :, seq_off + Q0 : seq_off + Q0 + W],
                            oT_ps[pr][:, :, :W],
                        )
                        nc.vector.tensor_copy(
                            sums_stage[pr][:, :, seq_off + Q0 : seq_off + Q0 + W],
                            sums_ps[pr][:, :, :W],
                        )
                # stream this sequence's outputs while later seqs compute
                for pr in range(2):
                    nc.sync.dma_start(
                        oT_ext[pr][:, :, seq_off : seq_off + Ls],
                        ot_stage[pr][:, :, seq_off : seq_off + Ls],
                    )
                seq_off += Ls

            for pr in range(2):
                nc.sync.dma_start(sums_ext[pr][:], sums_stage[pr][:])

    nc.finalize()
    return nc


def _install_ntff_hook():
    """Shim antenv.axon_hooks (absent in this container) so trace=True can
    reach the terminal's NRT profiler via libaxon_pjrt.so ctypes."""
    import types

    if "antenv.axon_hooks" in sys.modules:
        return
    import antenv
    from concourse import bass_utils

    mod = types.ModuleType("antenv.axon_hooks")
    state = {"hook": None}
    mod.set_axon_ntff_profile_hook = lambda h: state.__setitem__("hook", h)
    mod.get_axon_ntff_profile_hook = lambda: state["hook"]
    sys.modules["antenv.axon_hooks"] = mod
    antenv.axon_hooks = mod
    bass_utils.upload_artifacts = lambda tmpdir: tmpdir  # zero-egress container
    try:
        if "/root/.axon_site" not in sys.path:
            sys.path.insert(0, "/root/.axon_site")
        from trn_agent_boot.trn_boot import _ntff_profile_via_ctypes

        mod.set_axon_ntff_profile_hook(
            _ntff_profile_via_ctypes("/opt/axon/libaxon_pjrt.so")
        )
    except Exception:
        pass


def kernel(q, k, v, cu_seqlens, max_seqlen):
    from concourse import bass_utils

    q = np.asarray(q, dtype=np.float32)
    k = np.asarray(k, dtype=np.float32)
    v = np.asarray(v, dtype=np.float32)
    cu = np.asarray(cu_seqlens, dtype=np.int64)
    T_host = q.shape[0]
    lengths = np.diff(cu).astype(np.int64)
    all_nblocks = [int((L + BLK - 1) // BLK) for L in lengths]
    T_pad = sum(all_nblocks) * BLK

    # process sequences longest-first: big seq warms the pipeline while the
    # rest of the data streams in, and the tail drains a small seq
    order = sorted(range(len(lengths)), key=lambda s: -all_nblocks[s])
    nblocks = [all_nblocks[s] for s in order]

    # host -> padded device token index map (valid tokens only)
    dev_idx = np.zeros(T_host, dtype=np.int64)
    pad_off = 0
    for s in order:
        L = int(lengths[s])
        dev_idx[cu[s] : cu[s] + L] = pad_off + np.arange(L)
        pad_off += all_nblocks[s] * BLK

    bf16 = ml_dtypes.bfloat16
    qp = np.zeros((T_pad, NUM_HEADS * HEAD_DIM), bf16)
    kp = np.zeros((T_pad, NUM_KV_HEADS * HEAD_DIM), bf16)
    vp = np.zeros((T_pad, NUM_KV_HEADS * HEAD_DIM), bf16)
    qp[dev_idx] = q.astype(bf16)
    kp[dev_idx] = k.astype(bf16)
    vp[dev_idx] = v.astype(bf16)

    mask1 = np.where(
        np.arange(BLK)[:, None] <= np.arange(BLK)[None, :], 1.0, 0.0
    ).astype(bf16)
    mask = np.broadcast_to(
        mask1[:, None, :], (BLK, HEADS_PER_CORE, BLK)
    ).copy()

    key = tuple(nblocks)
    if key not in _GRAPH_CACHE:
        _GRAPH_CACHE[key] = _build_graph(key)
    nc = _GRAPH_CACHE[key]

    in_maps = []
    for c in range(N_CORES):
        m = {"mask": mask}
        m["kT"] = np.ascontiguousarray(kp[:, c * HEAD_DIM : (c + 1) * HEAD_DIM].T)
        m["v"] = np.ascontiguousarray(vp[:, c * HEAD_DIM : (c + 1) * HEAD_DIM])
        # [d, h, t] head-interleaved Q^T so head pairs are 3D-AP adjacent
        qc = qp[:, c * HEADS_PER_CORE * HEAD_DIM : (c + 1) * HEADS_PER_CORE * HEAD_DIM]
        m["qT"] = np.ascontiguousarray(
            qc.reshape(T_pad, HEADS_PER_CORE, HEAD_DIM).transpose(2, 1, 0)
        )
        in_maps.append(m)

    trace = bool(os.environ.get("BASS_TRACE"))
    if trace:
        _install_ntff_hook()
    res = bass_utils.run_bass_kernel_spmd(
        nc, in_maps, core_ids=list(range(N_CORES)), trace=trace
    )
    if trace and res.exec_time_ns is not None:
        print(f"HW exec time: {res.exec_time_ns} ns")
        if res.instructions_and_trace is not None:
            print(f"trace: {res.instructions_and_trace[1]}")

    out = np.empty((T_host, NUM_HEADS * HEAD_DIM), np.float32)
    for c in range(N_CORES):
        r = res.results[c]
        for h in range(HEADS_PER_CORE):
            gh = c * HEADS_PER_CORE + h
            oT = r[f"oT{h // 2}"][:, h % 2]  # [128, T_pad] unnormalized
            sums = r[f"sums{h // 2}"][0, h % 2]  # [T_pad]
            o = (oT[:, dev_idx] / sums[dev_idx][None, :]).T  # [T_host, 128]
            out[:, gh * HEAD_DIM : (gh + 1) * HEAD_DIM] = o
    return out


# revision 33
# speedup vs baseline: 1.0654x; 1.0107x over previous
"""Varlen causal GQA attention on 8 TRN2 NeuronCores.

Problem: 32 q heads, 8 kv heads, head_dim 128, ragged batch (cu_seqlens),
f32. Sharded by KV-head group: core c owns kv head c and q heads
4c..4c+3 — fully data-independent across cores, no collectives.

Per core, blockwise causal attention with all 4 q heads fused via 3D
access patterns (q stored head-interleaved [d, h, t]):
    S^T[h][k, q] = (K_j)^T.T @ Q^T[h]   2 matmuls (head pairs), shared K_j
    P = exp(S * scale)                  ONE ScalarE op over all 4 heads
    causal mask: 0/1 multiply on GpSimd (off the ACT/DVE critical path)
    O^T[h] += V_j @ P[h]                2 matmuls (head pairs), shared V_j
    sums[h] += ones.T @ P[h]            2 M=1 matmuls (head pairs)
Each PSUM bank carries exactly one accumulation chain (head pairs share
a bank through a single 3D-AP matmul). Host does all transposes
(Q^T/K^T in, O^T -> O out), bf16 conversion, and the softmax division.
q groups are 256 wide so PSUM fits: S4 (2 banks x2) + O pairs (2) +
sums pairs (2) = 8 banks.
"""

import math
import os
import sys

sys.path.insert(0, "/opt/trn_rl_repo")

import ml_dtypes
import numpy as np

NUM_HEADS = 32
NUM_KV_HEADS = 8
HEAD_DIM = 128
HEADS_PER_CORE = NUM_HEADS // NUM_KV_HEADS  # 4
N_CORES = 8
BLK = 128
GROUP = 256
SCALE = 1.0 / math.sqrt(HEAD_DIM)

_GRAPH_CACHE = {}


def _build_graph(seq_blocks):
    """Build the SPMD Bacc graph for padded per-seq block counts."""
    from concourse import bacc
    import concourse.mybir as mybir
    from concourse.tile import TileContext

    f32 = mybir.dt.float32
    bf16 = mybir.dt.bfloat16
    T = sum(seq_blocks) * BLK
    n_blocks_total = T // BLK
    H = HEADS_PER_CORE

    nc = bacc.Bacc("TRN2", target_bir_lowering=False, debug=False,
                   num_devices=N_CORES)

    qT_ext = nc.declare_dram_parameter("qT", [BLK, H, T], bf16, isOutput=False)
    kT_ext = nc.declare_dram_parameter("kT", [BLK, T], bf16, isOutput=False)
    v_ext = nc.declare_dram_parameter("v", [T, HEAD_DIM], bf16, isOutput=False)
    mask_ext = nc.declare_dram_parameter("mask", [BLK, H, BLK], bf16,
                                         isOutput=False)
    oT_ext = [
        nc.declare_dram_parameter(f"oT{pr}", [BLK, 2, T], f32, isOutput=True)
        for pr in range(2)
    ]
    sums_ext = [
        nc.declare_dram_parameter(f"sums{pr}", [1, 2, T], f32, isOutput=True)
        for pr in range(2)
    ]

    with TileContext(nc) as tc:
        with (
            tc.tile_pool(name="persist", bufs=1) as persist,
            tc.tile_pool(name="p", bufs=8) as p_pool,
            tc.tile_pool(name="ps_s4", bufs=2, space="PSUM") as ps_s4,
            tc.tile_pool(name="ps_o", bufs=1, space="PSUM") as ps_o,
            tc.tile_pool(name="ps_sum", bufs=1, space="PSUM") as ps_sum,
        ):
            kT_sb = persist.tile([BLK, T], bf16)
            v_sb = persist.tile([BLK, n_blocks_total, HEAD_DIM], bf16)
            mask_sb = persist.tile([BLK, H, BLK], bf16)
            qT_sb = persist.tile([BLK, H, T], bf16)
            v_re = v_ext[:].rearrange("(j p) d -> p j d", p=BLK)
            nb0 = seq_blocks[0]
            # micro-chunks for the very first group's data, then the rest of
            # seq 0, so the first matmul fires as early as possible
            c00 = min(4 * BLK, nb0 * BLK)
            nc.sync.dma_start(kT_sb[:, :c00], kT_ext[:, :c00])
            nc.sync.dma_start(qT_sb[:, :, :c00], qT_ext[:, :, :c00])
            nc.sync.dma_start(v_sb[:, : c00 // BLK, :], v_re[:, : c00 // BLK, :])
            if c00 < nb0 * BLK:
                nc.sync.dma_start(kT_sb[:, c00 : nb0 * BLK], kT_ext[:, c00 : nb0 * BLK])
                nc.sync.dma_start(
                    qT_sb[:, :, c00 : nb0 * BLK], qT_ext[:, :, c00 : nb0 * BLK]
                )
                nc.sync.dma_start(
                    v_sb[:, c00 // BLK : nb0, :], v_re[:, c00 // BLK : nb0, :]
                )
            nc.scalar.dma_start(mask_sb[:], mask_ext[:])
            if nb0 < n_blocks_total:
                c0 = nb0 * BLK
                nc.scalar.dma_start(kT_sb[:, c0:], kT_ext[:, c0:])
                nc.scalar.dma_start(v_sb[:, nb0:, :], v_re[:, nb0:, :])
                nc.sync.dma_start(qT_sb[:, :, c0:], qT_ext[:, :, c0:])

            ones_f = persist.tile([BLK, 1], f32)
            nc.vector.memset(ones_f[:], 1.0)
            ones_b = persist.tile([BLK, 1], bf16)
            nc.vector.tensor_copy(ones_b[:], ones_f[:])

            ot_stage = [
                persist.tile([BLK, 2, T], f32, name=f"ot_stage{pr}")
                for pr in range(2)
            ]
            sums_stage = [
                persist.tile([1, 2, T], f32, name=f"sums_stage{pr}")
                for pr in range(2)
            ]

            seq_off = 0
            for nblk in seq_blocks:
                Ls = nblk * BLK
                for g in range((Ls + GROUP - 1) // GROUP):
                    Q0 = g * GROUP
                    W = min(GROUP, Ls - Q0)
                    jmax = (Q0 + W) // BLK - 1
                    # one PSUM bank per head pair, one acc chain per bank
                    oT_ps = [
                        ps_o.tile([BLK, 2, GROUP], f32, tag=f"ot_ps{pr}",
                                  name="oT_ps")
                        for pr in range(2)
                    ]
                    sums_ps = [
                        ps_sum.tile([1, 2, GROUP], f32, tag=f"sums_ps{pr}",
                                    name="sums_ps")
                        for pr in range(2)
                    ]
                    # j descending: diagonal (masked) blocks issue first so
                    # their GpSimd mask latency hides under the full blocks
                    j_first = jmax
                    for j in range(jmax, -1, -1):
                        cs = max(0, BLK * j - Q0)
                        N = W - cs
                        kj = kT_sb[:, seq_off + j * BLK : seq_off + (j + 1) * BLK]
                        vj = v_sb[:, seq_off // BLK + j, :]
                        s4 = ps_s4.tile([BLK, H, GROUP], f32, tag="s4",
                                        name="s4")
                        for pr in range(2):
                            nc.tensor.matmul(
                                s4[:, 2 * pr : 2 * pr + 2, cs : cs + N],
                                kj,
                                qT_sb[
                                    :,
                                    2 * pr : 2 * pr + 2,
                                    seq_off + Q0 + cs : seq_off + Q0 + cs + N,
                                ],
                                start=True,
                                stop=True,
                            )
                        p4 = p_pool.tile([BLK, H, GROUP], bf16, tag="p4",
                                         name="p4")
                        nc.scalar.activation(
                            p4[:, :, :N],
                            s4[:, :, cs : cs + N],
                            mybir.ActivationFunctionType.Exp,
                            scale=SCALE,
                        )
                        if BLK * j >= Q0:  # diagonal: zero upper triangle
                            nc.gpsimd.tensor_mul(
                                p4[:, :, :BLK], p4[:, :, :BLK], mask_sb[:]
                            )
                        for pr in range(2):
                            nc.tensor.matmul(
                                oT_ps[pr][:, :, cs : cs + N],
                                vj,
                                p4[:, 2 * pr : 2 * pr + 2, :N],
                                start=(j == j_first),
                                stop=(j == 0),
                            )
                        for pr in range(2):
                            nc.tensor.matmul(
                                sums_ps[pr][:, :, cs : cs + N],
                                ones_b[:],
                                p4[:, 2 * pr : 2 * pr + 2, :N],
                                start=(j == j_first),
                                stop=(j == 0),
                            )
                    for pr in range(2):
                        nc.vector.tensor_copy(
                            ot_stage[pr][:, :, seq_off + Q0 : seq_off + Q0 + W],
                            oT_ps[pr][:, :, :W],
                        )
                        nc.vector.tensor_copy(
                            sums_stage[pr][:, :, seq_off + Q0 : seq_off + Q0 + W],
                            sums_ps[pr][:, :, :W],
                        )
                # stream this sequence's outputs while later seqs compute;
                # for the final sequence split per group to shrink the tail
                if seq_off + Ls == T:
                    for pr in range(2):
                        nc.sync.dma_start(sums_ext[pr][:], sums_stage[pr][:])
                    for g0 in range(0, Ls, GROUP):
                        W0 = min(GROUP, Ls - g0)
                        for pr in range(2):
                            nc.sync.dma_start(
                                oT_ext[pr][:, :, seq_off + g0 : seq_off + g0 + W0],
                                ot_stage[pr][:, :, seq_off + g0 : seq_off + g0 + W0],
                            )
                else:
                    for pr in range(2):
                        nc.sync.dma_start(
                            oT_ext[pr][:, :, seq_off : seq_off + Ls],
                            ot_stage[pr][:, :, seq_off : seq_off + Ls],
                        )
                seq_off += Ls

    nc.finalize()
    return nc


def _install_ntff_hook():
    """Shim antenv.axon_hooks (absent in this container) so trace=True can
    reach the terminal's NRT profiler via libaxon_pjrt.so ctypes."""
    import types

    if "antenv.axon_hooks" in sys.modules:
        return
    import antenv
    from concourse import bass_utils

    mod = types.ModuleType("antenv.axon_hooks")
    state = {"hook": None}
    mod.set_axon_ntff_profile_hook = lambda h: state.__setitem__("hook", h)
    mod.get_axon_ntff_profile_hook = lambda: state["hook"]
    sys.modules["antenv.axon_hooks"] = mod
    antenv.axon_hooks = mod
    bass_utils.upload_artifacts = lambda tmpdir: tmpdir  # zero-egress container
    try:
        if "/root/.axon_site" not in sys.path:
            sys.path.insert(0, "/root/.axon_site")
        from trn_agent_boot.trn_boot import _ntff_profile_via_ctypes

        mod.set_axon_ntff_profile_hook(
            _ntff_profile_via_ctypes("/opt/axon/libaxon_pjrt.so")
        )
    except Exception:
        pass


def kernel(q, k, v, cu_seqlens, max_seqlen):
    from concourse import bass_utils

    q = np.asarray(q, dtype=np.float32)
    k = np.asarray(k, dtype=np.float32)
    v = np.asarray(v, dtype=np.float32)
    cu = np.asarray(cu_seqlens, dtype=np.int64)
    T_host = q.shape[0]
    lengths = np.diff(cu).astype(np.int64)
    all_nblocks = [int((L + BLK - 1) // BLK) for L in lengths]
    T_pad = sum(all_nblocks) * BLK

    # process sequences longest-first: big seq warms the pipeline while the
    # rest of the data streams in, and the tail drains a small seq
    order = sorted(range(len(lengths)), key=lambda s: -all_nblocks[s])
    nblocks = [all_nblocks[s] for s in order]

    # host -> padded device token index map (valid tokens only)
    dev_idx = np.zeros(T_host, dtype=np.int64)
    pad_off = 0
    for s in order:
        L = int(lengths[s])
        dev_idx[cu[s] : cu[s] + L] = pad_off + np.arange(L)
        pad_off += all_nblocks[s] * BLK

    bf16 = ml_dtypes.bfloat16
    qp = np.zeros((T_pad, NUM_HEADS * HEAD_DIM), bf16)
    kp = np.zeros((T_pad, NUM_KV_HEADS * HEAD_DIM), bf16)
    vp = np.zeros((T_pad, NUM_KV_HEADS * HEAD_DIM), bf16)
    qp[dev_idx] = q.astype(bf16)
    kp[dev_idx] = k.astype(bf16)
    vp[dev_idx] = v.astype(bf16)

    mask1 = np.where(
        np.arange(BLK)[:, None] <= np.arange(BLK)[None, :], 1.0, 0.0
    ).astype(bf16)
    mask = np.broadcast_to(
        mask1[:, None, :], (BLK, HEADS_PER_CORE, BLK)
    ).copy()

    key = tuple(nblocks)
    if key not in _GRAPH_CACHE:
        _GRAPH_CACHE[key] = _build_graph(key)
    nc = _GRAPH_CACHE[key]

    in_maps = []
    for c in range(N_CORES):
        m = {"mask": mask}
        m["kT"] = np.ascontiguousarray(kp[:, c * HEAD_DIM : (c + 1) * HEAD_DIM].T)
        m["v"] = np.ascontiguousarray(vp[:, c * HEAD_DIM : (c + 1) * HEAD_DIM])
        # [d, h, t] head-interleaved Q^T so head pairs are 3D-AP adjacent
        qc = qp[:, c * HEADS_PER_CORE * HEAD_DIM : (c + 1) * HEADS_PER_CORE * HEAD_DIM]
        m["qT"] = np.ascontiguousarray(
            qc.reshape(T_pad, HEADS_PER_CORE, HEAD_DIM).transpose(2, 1, 0)
        )
        in_maps.append(m)

    trace = bool(os.environ.get("BASS_TRACE"))
    if trace:
        _install_ntff_hook()
    res = bass_utils.run_bass_kernel_spmd(
        nc, in_maps, core_ids=list(range(N_CORES)), trace=trace
    )
    if trace and res.exec_time_ns is not None:
        print(f"HW exec time: {res.exec_time_ns} ns")
        if res.instructions_and_trace is not None:
            print(f"trace: {res.instructions_and_trace[1]}")

    out = np.empty((T_host, NUM_HEADS * HEAD_DIM), np.float32)
    for c in range(N_CORES):
        r = res.results[c]
        for h in range(HEADS_PER_CORE):
            gh = c * HEADS_PER_CORE + h
            oT = r[f"oT{h // 2}"][:, h % 2]  # [128, T_pad] unnormalized
            sums = r[f"sums{h // 2}"][0, h % 2]  # [T_pad]
            o = (oT[:, dev_idx] / sums[dev_idx][None, :]).T  # [T_host, 128]
            out[:, gh * HEAD_DIM : (gh + 1) * HEAD_DIM] = o
    return out


# revision 34
# speedup vs baseline: 1.1223x; 1.0534x over previous
"""Varlen causal GQA attention on 8 TRN2 NeuronCores.

Problem: 32 q heads, 8 kv heads, head_dim 128, ragged batch (cu_seqlens),
f32. Sharded by KV-head group: core c owns kv head c and q heads
4c..4c+3 — fully data-independent across cores, no collectives.

Per core, blockwise causal attention in 128x128 blocks with all 4 q
heads fused through 3D access patterns (q stored head-interleaved
[d, h, t]), so every matmul streams exactly 4*128 = 512 columns and
each PSUM bank carries exactly one accumulation chain:
    S[k, h, q]  = (K_j)^T.T @ Q^T      ONE matmul per (q-block, k-block)
    P = exp(S * scale)                 ONE ScalarE op
    causal mask: 0/1 multiply on GpSimd (diagonal blocks only)
    O^T[h] += V_j @ P                  ONE matmul, PSUM-accumulated over j
    sums[h] += ones.T @ P              ONE M=1 matmul
Host does all transposes (Q^T/K^T in, O^T -> O out), bf16 conversion,
and the final softmax division.
"""

import math
import os
import sys

sys.path.insert(0, "/opt/trn_rl_repo")

import ml_dtypes
import numpy as np

NUM_HEADS = 32
NUM_KV_HEADS = 8
HEAD_DIM = 128
HEADS_PER_CORE = NUM_HEADS // NUM_KV_HEADS  # 4
N_CORES = 8
BLK = 128
SCALE = 1.0 / math.sqrt(HEAD_DIM)

_GRAPH_CACHE = {}


def _build_graph(seq_blocks):
    """Build the SPMD Bacc graph for padded per-seq block counts."""
    from concourse import bacc
    import concourse.mybir as mybir
    from concourse.tile import TileContext

    f32 = mybir.dt.float32
    bf16 = mybir.dt.bfloat16
    T = sum(seq_blocks) * BLK
    n_blocks_total = T // BLK
    H = HEADS_PER_CORE

    nc = bacc.Bacc("TRN2", target_bir_lowering=False, debug=False,
                   num_devices=N_CORES)

    qT_ext = nc.declare_dram_parameter("qT", [BLK, H, T], bf16, isOutput=False)
    kT_ext = nc.declare_dram_parameter("kT", [BLK, T], bf16, isOutput=False)
    v_ext = nc.declare_dram_parameter("v", [T, HEAD_DIM], bf16, isOutput=False)
    mask_ext = nc.declare_dram_parameter("mask", [BLK, H, BLK], bf16,
                                         isOutput=False)
    oT_ext = nc.declare_dram_parameter("oT", [BLK, H, T], f32, isOutput=True)
    sums_ext = nc.declare_dram_parameter("sums", [1, H, T], f32, isOutput=True)

    with TileContext(nc) as tc:
        with (
            tc.tile_pool(name="persist", bufs=1) as persist,
            tc.tile_pool(name="p", bufs=8) as p_pool,
            tc.tile_pool(name="ps_s4", bufs=4, space="PSUM") as ps_s4,
            tc.tile_pool(name="ps_o", bufs=2, space="PSUM") as ps_o,
            tc.tile_pool(name="ps_sum", bufs=2, space="PSUM") as ps_sum,
        ):
            kT_sb = persist.tile([BLK, T], bf16)
            v_sb = persist.tile([BLK, n_blocks_total, HEAD_DIM], bf16)
            mask_sb = persist.tile([BLK, H, BLK], bf16)
            qT_sb = persist.tile([BLK, H, T], bf16)
            v_re = v_ext[:].rearrange("(j p) d -> p j d", p=BLK)
            nb0 = seq_blocks[0]
            # micro-chunks for the very first blocks' data, then the rest of
            # seq 0, so the first matmul fires as early as possible
            c00 = min(4 * BLK, nb0 * BLK)
            nc.sync.dma_start(kT_sb[:, :c00], kT_ext[:, :c00])
            nc.sync.dma_start(qT_sb[:, :, :c00], qT_ext[:, :, :c00])
            nc.sync.dma_start(v_sb[:, : c00 // BLK, :], v_re[:, : c00 // BLK, :])
            if c00 < nb0 * BLK:
                nc.sync.dma_start(kT_sb[:, c00 : nb0 * BLK],
                                  kT_ext[:, c00 : nb0 * BLK])
                nc.sync.dma_start(qT_sb[:, :, c00 : nb0 * BLK],
                                  qT_ext[:, :, c00 : nb0 * BLK])
                nc.sync.dma_start(v_sb[:, c00 // BLK : nb0, :],
                                  v_re[:, c00 // BLK : nb0, :])
            nc.scalar.dma_start(mask_sb[:], mask_ext[:])
            if nb0 < n_blocks_total:
                c0 = nb0 * BLK
                nc.scalar.dma_start(kT_sb[:, c0:], kT_ext[:, c0:])
                nc.scalar.dma_start(v_sb[:, nb0:, :], v_re[:, nb0:, :])
                nc.sync.dma_start(qT_sb[:, :, c0:], qT_ext[:, :, c0:])

            ones_f = persist.tile([BLK, 1], f32)
            nc.vector.memset(ones_f[:], 1.0)
            ones_b = persist.tile([BLK, 1], bf16)
            nc.vector.tensor_copy(ones_b[:], ones_f[:])

            ot_stage = persist.tile([BLK, H, T], f32)
            sums_stage = persist.tile([1, H, T], f32)

            seq_off = 0
            for nblk in seq_blocks:
                Ls = nblk * BLK
                for g in range(nblk):
                    Q0 = seq_off + g * BLK
                    oT_ps = ps_o.tile([BLK, H, BLK], f32, tag="ot_ps",
                                      name="oT_ps")
                    sums_ps = ps_sum.tile([1, H, BLK], f32, tag="sums_ps",
                                          name="sums_ps")
                    # j descending: the diagonal (masked) block issues first
                    # so its GpSimd mask latency hides under the full blocks
                    for j in range(g, -1, -1):
                        kj = kT_sb[:, seq_off + j * BLK : seq_off + (j + 1) * BLK]
                        vj = v_sb[:, seq_off // BLK + j, :]
                        s4 = ps_s4.tile([BLK, H, BLK], f32, tag="s4",
                                        name="s4")
                        nc.tensor.matmul(
                            s4[:],
                            kj,
                            qT_sb[:, :, Q0 : Q0 + BLK],
                            start=True,
                            stop=True,
                        )
                        p4 = p_pool.tile([BLK, H, BLK], bf16, tag="p4",
                                         name="p4")
                        nc.scalar.activation(
                            p4[:],
                            s4[:],
                            mybir.ActivationFunctionType.Exp,
                            scale=SCALE,
                        )
                        if j == g:  # diagonal: zero the upper triangle
                            nc.gpsimd.tensor_mul(p4[:], p4[:], mask_sb[:])
                        nc.tensor.matmul(
                            oT_ps[:],
                            vj,
                            p4[:],
                            start=(j == g),
                            stop=(j == 0),
                        )
                        nc.tensor.matmul(
                            sums_ps[:],
                            ones_b[:],
                            p4[:],
                            start=(j == g),
                            stop=(j == 0),
                        )
                    nc.vector.tensor_copy(
                        ot_stage[:, :, Q0 : Q0 + BLK], oT_ps[:]
                    )
                    nc.vector.tensor_copy(
                        sums_stage[:, :, Q0 : Q0 + BLK], sums_ps[:]
                    )
                # stream this sequence's outputs while later seqs compute;
                # for the final sequence split finer to shrink the tail
                if seq_off + Ls == T:
                    nc.sync.dma_start(sums_ext[:], sums_stage[:])
                    for g0 in range(0, Ls, 2 * BLK):
                        W0 = min(2 * BLK, Ls - g0)
                        nc.sync.dma_start(
                            oT_ext[:, :, seq_off + g0 : seq_off + g0 + W0],
                            ot_stage[:, :, seq_off + g0 : seq_off + g0 + W0],
                        )
                else:
                    nc.sync.dma_start(
                        oT_ext[:, :, seq_off : seq_off + Ls],
                        ot_stage[:, :, seq_off : seq_off + Ls],
                    )
                seq_off += Ls

    nc.finalize()
    return nc


def _install_ntff_hook():
    """Shim antenv.axon_hooks (absent in this container) so trace=True can
    reach the terminal's NRT profiler via libaxon_pjrt.so ctypes."""
    import types

    if "antenv.axon_hooks" in sys.modules:
        return
    import antenv
    from concourse import bass_utils

    mod = types.ModuleType("antenv.axon_hooks")
    state = {"hook": None}
    mod.set_axon_ntff_profile_hook = lambda h: state.__setitem__("hook", h)
    mod.get_axon_ntff_profile_hook = lambda: state["hook"]
    sys.modules["antenv.axon_hooks"] = mod
    antenv.axon_hooks = mod
    bass_utils.upload_artifacts = lambda tmpdir: tmpdir  # zero-egress container
    try:
        if "/root/.axon_site" not in sys.path:
            sys.path.insert(0, "/root/.axon_site")
        from trn_agent_boot.trn_boot import _ntff_profile_via_ctypes

        mod.set_axon_ntff_profile_hook(
            _ntff_profile_via_ctypes("/opt/axon/libaxon_pjrt.so")
        )
    except Exception:
        pass


def kernel(q, k, v, cu_seqlens, max_seqlen):
    from concourse import bass_utils

    q = np.asarray(q, dtype=np.float32)
    k = np.asarray(k, dtype=np.float32)
    v = np.asarray(v, dtype=np.float32)
    cu = np.asarray(cu_seqlens, dtype=np.int64)
    T_host = q.shape[0]
    lengths = np.diff(cu).astype(np.int64)
    all_nblocks = [int((L + BLK - 1) // BLK) for L in lengths]
    T_pad = sum(all_nblocks) * BLK

    # process sequences longest-first: big seq warms the pipeline while the
    # rest of the data streams in, and the tail drains a small seq
    order = sorted(range(len(lengths)), key=lambda s: -all_nblocks[s])
    nblocks = [all_nblocks[s] for s in order]

    # host -> padded device token index map (valid tokens only)
    dev_idx = np.zeros(T_host, dtype=np.int64)
    pad_off = 0
    for s in order:
        L = int(lengths[s])
        dev_idx[cu[s] : cu[s] + L] = pad_off + np.arange(L)
        pad_off += all_nblocks[s] * BLK

    bf16 = ml_dtypes.bfloat16
    qp = np.zeros((T_pad, NUM_HEADS * HEAD_DIM), bf16)
    kp = np.zeros((T_pad, NUM_KV_HEADS * HEAD_DIM), bf16)
    vp = np.zeros((T_pad, NUM_KV_HEADS * HEAD_DIM), bf16)
    qp[dev_idx] = q.astype(bf16)
    kp[dev_idx] = k.astype(bf16)
    vp[dev_idx] = v.astype(bf16)

    mask1 = np.where(
        np.arange(BLK)[:, None] <= np.arange(BLK)[None, :], 1.0, 0.0
    ).astype(bf16)
    mask = np.broadcast_to(
        mask1[:, None, :], (BLK, HEADS_PER_CORE, BLK)
    ).copy()

    key = tuple(nblocks)
    if key not in _GRAPH_CACHE:
        _GRAPH_CACHE[key] = _build_graph(key)
    nc = _GRAPH_CACHE[key]

    in_maps = []
    for c in range(N_CORES):
        m = {"mask": mask}
        m["kT"] = np.ascontiguousarray(kp[:, c * HEAD_DIM : (c + 1) * HEAD_DIM].T)
        m["v"] = np.ascontiguousarray(vp[:, c * HEAD_DIM : (c + 1) * HEAD_DIM])
        # [d, h, t] head-interleaved Q^T so all 4 heads ride one 3D AP
        qc = qp[:, c * HEADS_PER_CORE * HEAD_DIM : (c + 1) * HEADS_PER_CORE * HEAD_DIM]
        m["qT"] = np.ascontiguousarray(
            qc.reshape(T_pad, HEADS_PER_CORE, HEAD_DIM).transpose(2, 1, 0)
        )
        in_maps.append(m)

    trace = bool(os.environ.get("BASS_TRACE"))
    if trace:
        _install_ntff_hook()
    res = bass_utils.run_bass_kernel_spmd(
        nc, in_maps, core_ids=list(range(N_CORES)), trace=trace
    )
    if trace and res.exec_time_ns is not None:
        print(f"HW exec time: {res.exec_time_ns} ns")
        if res.instructions_and_trace is not None:
            print(f"trace: {res.instructions_and_trace[1]}")

    out = np.empty((T_host, NUM_HEADS * HEAD_DIM), np.float32)
    for c in range(N_CORES):
        r = res.results[c]
        oT = r["oT"]  # [128, H, T_pad] unnormalized
        sums = r["sums"][0]  # [H, T_pad]
        for h in range(HEADS_PER_CORE):
            gh = c * HEADS_PER_CORE + h
            o = (oT[:, h][:, dev_idx] / sums[h][dev_idx][None, :]).T
            out[:, gh * HEAD_DIM : (gh + 1) * HEAD_DIM] = o
    return out


# revision 35
# speedup vs baseline: 1.1324x; 1.0090x over previous
"""Varlen causal GQA attention on 8 TRN2 NeuronCores.

Problem: 32 q heads, 8 kv heads, head_dim 128, ragged batch (cu_seqlens),
f32. Sharded by KV-head group: core c owns kv head c and q heads
4c..4c+3 — fully data-independent across cores, no collectives.

Per core, blockwise causal attention in 128x128 blocks with all 4 q
heads fused through 3D access patterns (q stored head-interleaved
[d, h, t]), so every matmul streams exactly 4*128 = 512 columns and
each PSUM bank carries exactly one accumulation chain:
    S[k, h, q]  = (K_j)^T.T @ Q^T      ONE matmul per (q-block, k-block)
    P = exp(S * scale)                 ONE ScalarE op
    causal mask: 0/1 multiply on GpSimd (diagonal blocks only)
    O^T[h] += V_j @ P                  ONE matmul, PSUM-accumulated over j
    sums[h] += ones.T @ P              ONE M=1 matmul
Host does all transposes (Q^T/K^T in, O^T -> O out), bf16 conversion,
and the final softmax division.
"""

import math
import os
import sys

sys.path.insert(0, "/opt/trn_rl_repo")

import ml_dtypes
import numpy as np

NUM_HEADS = 32
NUM_KV_HEADS = 8
HEAD_DIM = 128
HEADS_PER_CORE = NUM_HEADS // NUM_KV_HEADS  # 4
N_CORES = 8
BLK = 128
SCALE = 1.0 / math.sqrt(HEAD_DIM)

_GRAPH_CACHE = {}


def _build_graph(seq_blocks):
    """Build the SPMD Bacc graph for padded per-seq block counts."""
    from concourse import bacc
    import concourse.mybir as mybir
    from concourse.tile import TileContext

    f32 = mybir.dt.float32
    bf16 = mybir.dt.bfloat16
    T = sum(seq_blocks) * BLK
    n_blocks_total = T // BLK
    H = HEADS_PER_CORE

    nc = bacc.Bacc("TRN2", target_bir_lowering=False, debug=False,
                   num_devices=N_CORES)

    qT_ext = nc.declare_dram_parameter("qT", [BLK, H, T], bf16, isOutput=False)
    kT_ext = nc.declare_dram_parameter("kT", [BLK, T], bf16, isOutput=False)
    v_ext = nc.declare_dram_parameter("v", [T, HEAD_DIM], bf16, isOutput=False)
    mask_ext = nc.declare_dram_parameter("mask", [BLK, H, BLK], bf16,
                                         isOutput=False)
    oT_ext = nc.declare_dram_parameter("oT", [BLK, H, T], f32, isOutput=True)
    sums_ext = nc.declare_dram_parameter("sums", [1, H, T], f32, isOutput=True)

    with TileContext(nc) as tc:
        with (
            tc.tile_pool(name="persist", bufs=1) as persist,
            tc.tile_pool(name="p", bufs=8) as p_pool,
            tc.tile_pool(name="ps_s4", bufs=4, space="PSUM") as ps_s4,
            tc.tile_pool(name="ps_o", bufs=2, space="PSUM") as ps_o,
            tc.tile_pool(name="ps_sum", bufs=2, space="PSUM") as ps_sum,
        ):
            kT_sb = persist.tile([BLK, T], bf16)
            v_sb = persist.tile([BLK, n_blocks_total, HEAD_DIM], bf16)
            mask_sb = persist.tile([BLK, H, BLK], bf16)
            qT_sb = persist.tile([BLK, H, T], bf16)
            v_re = v_ext[:].rearrange("(j p) d -> p j d", p=BLK)
            nb0 = seq_blocks[0]
            # micro-chunks for the very first blocks' data, then the rest of
            # seq 0, so the first matmul fires as early as possible
            c00 = min(4 * BLK, nb0 * BLK)
            nc.sync.dma_start(kT_sb[:, :c00], kT_ext[:, :c00])
            nc.sync.dma_start(qT_sb[:, :, :c00], qT_ext[:, :, :c00])
            nc.sync.dma_start(v_sb[:, : c00 // BLK, :], v_re[:, : c00 // BLK, :])
            if c00 < nb0 * BLK:
                nc.sync.dma_start(kT_sb[:, c00 : nb0 * BLK],
                                  kT_ext[:, c00 : nb0 * BLK])
                nc.sync.dma_start(qT_sb[:, :, c00 : nb0 * BLK],
                                  qT_ext[:, :, c00 : nb0 * BLK])
                nc.sync.dma_start(v_sb[:, c00 // BLK : nb0, :],
                                  v_re[:, c00 // BLK : nb0, :])
            nc.scalar.dma_start(mask_sb[:], mask_ext[:])
            if nb0 < n_blocks_total:
                c0 = nb0 * BLK
                nc.scalar.dma_start(kT_sb[:, c0:], kT_ext[:, c0:])
                nc.scalar.dma_start(v_sb[:, nb0:, :], v_re[:, nb0:, :])
                nc.sync.dma_start(qT_sb[:, :, c0:], qT_ext[:, :, c0:])

            ones_f = persist.tile([BLK, 1], f32)
            nc.vector.memset(ones_f[:], 1.0)
            ones_b = persist.tile([BLK, 1], bf16)
            nc.vector.tensor_copy(ones_b[:], ones_f[:])

            ot_stage = persist.tile([BLK, H, T], f32)
            sums_stage = persist.tile([1, H, T], f32)

            # flat iteration stream over (seq, q-block g, k-block j), with
            # j descending inside each group (diagonal/masked block first)
            items = []
            seq_off = 0
            for nblk in seq_blocks:
                for g in range(nblk):
                    for j in range(g, -1, -1):
                        items.append((seq_off, nblk, g, j))
                seq_off += nblk * BLK

            # software-pipelined ISSUE order: AV/sums trail their S/exp by
            # LAG iterations so the PE's FIFO never parks on an exp wait
            LAG = 2
            state = {}  # live group accumulators keyed by (seq_off, g)
            pending = []

            def emit_front(it):
                seq_off, nblk, g, j = it
                Q0 = seq_off + g * BLK
                kj = kT_sb[:, seq_off + j * BLK : seq_off + (j + 1) * BLK]
                s4 = ps_s4.tile([BLK, H, BLK], f32, tag="s4", name="s4")
                nc.tensor.matmul(
                    s4[:], kj, qT_sb[:, :, Q0 : Q0 + BLK],
                    start=True, stop=True,
                )
                p4 = p_pool.tile([BLK, H, BLK], bf16, tag="p4", name="p4")
                nc.scalar.activation(
                    p4[:], s4[:], mybir.ActivationFunctionType.Exp,
                    scale=SCALE,
                )
                if j == g:  # diagonal: zero the upper triangle
                    nc.gpsimd.tensor_mul(p4[:], p4[:], mask_sb[:])
                return p4

            def emit_back(it, p4):
                seq_off, nblk, g, j = it
                Q0 = seq_off + g * BLK
                key = (seq_off, g)
                if j == g:
                    state[key] = (
                        ps_o.tile([BLK, H, BLK], f32, tag="ot_ps",
                                  name="oT_ps"),
                        ps_sum.tile([1, H, BLK], f32, tag="sums_ps",
                                    name="sums_ps"),
                    )
                oT_ps, sums_ps = state[key]
                vj = v_sb[:, seq_off // BLK + j, :]
                nc.tensor.matmul(
                    oT_ps[:], vj, p4[:], start=(j == g), stop=(j == 0)
                )
                nc.tensor.matmul(
                    sums_ps[:], ones_b[:], p4[:], start=(j == g), stop=(j == 0)
                )
                if j == 0:
                    nc.vector.tensor_copy(
                        ot_stage[:, :, Q0 : Q0 + BLK], oT_ps[:]
                    )
                    nc.vector.tensor_copy(
                        sums_stage[:, :, Q0 : Q0 + BLK], sums_ps[:]
                    )
                    del state[key]
                    if g == nblk - 1:  # sequence finished: stream outputs
                        Ls = nblk * BLK
                        if seq_off + Ls == T:
                            nc.sync.dma_start(sums_ext[:], sums_stage[:])
                            for g0 in range(0, Ls, 2 * BLK):
                                W0 = min(2 * BLK, Ls - g0)
                                nc.sync.dma_start(
                                    oT_ext[:, :, seq_off + g0 : seq_off + g0 + W0],
                                    ot_stage[:, :, seq_off + g0 : seq_off + g0 + W0],
                                )
                        else:
                            nc.sync.dma_start(
                                oT_ext[:, :, seq_off : seq_off + Ls],
                                ot_stage[:, :, seq_off : seq_off + Ls],
                            )

            for it in items:
                p4 = emit_front(it)
                pending.append((it, p4))
                if len(pending) > LAG:
                    emit_back(*pending.pop(0))
            for it, p4 in pending:
                emit_back(it, p4)

    nc.finalize()
    return nc


def _install_ntff_hook():
    """Shim antenv.axon_hooks (absent in this container) so trace=True can
    reach the terminal's NRT profiler via libaxon_pjrt.so ctypes."""
    import types

    if "antenv.axon_hooks" in sys.modules:
        return
    import antenv
    from concourse import bass_utils

    mod = types.ModuleType("antenv.axon_hooks")
    state = {"hook": None}
    mod.set_axon_ntff_profile_hook = lambda h: state.__setitem__("hook", h)
    mod.get_axon_ntff_profile_hook = lambda: state["hook"]
    sys.modules["antenv.axon_hooks"] = mod
    antenv.axon_hooks = mod
    bass_utils.upload_artifacts = lambda tmpdir: tmpdir  # zero-egress container
    try:
        if "/root/.axon_site" not in sys.path:
            sys.path.insert(0, "/root/.axon_site")
        from trn_agent_boot.trn_boot import _ntff_profile_via_ctypes

        mod.set_axon_ntff_profile_hook(
            _ntff_profile_via_ctypes("/opt/axon/libaxon_pjrt.so")
        )
    except Exception:
        pass


def kernel(q, k, v, cu_seqlens, max_seqlen):
    from concourse import bass_utils

    q = np.asarray(q, dtype=np.float32)
    k = np.asarray(k, dtype=np.float32)
    v = np.asarray(v, dtype=np.float32)
    cu = np.asarray(cu_seqlens, dtype=np.int64)
    T_host = q.shape[0]
    lengths = np.diff(cu).astype(np.int64)
    all_nblocks = [int((L + BLK - 1) // BLK) for L in lengths]
    T_pad = sum(all_nblocks) * BLK

    # process sequences longest-first: big seq warms the pipeline while the
    # rest of the data streams in, and the tail drains a small seq
    order = sorted(range(len(lengths)), key=lambda s: -all_nblocks[s])
    nblocks = [all_nblocks[s] for s in order]

    # host -> padded device token index map (valid tokens only)
    dev_idx = np.zeros(T_host, dtype=np.int64)
    pad_off = 0
    for s in order:
        L = int(lengths[s])
        dev_idx[cu[s] : cu[s] + L] = pad_off + np.arange(L)
        pad_off += all_nblocks[s] * BLK

    bf16 = ml_dtypes.bfloat16
    qp = np.zeros((T_pad, NUM_HEADS * HEAD_DIM), bf16)
    kp = np.zeros((T_pad, NUM_KV_HEADS * HEAD_DIM), bf16)
    vp = np.zeros((T_pad, NUM_KV_HEADS * HEAD_DIM), bf16)
    qp[dev_idx] = q.astype(bf16)
    kp[dev_idx] = k.astype(bf16)
    vp[dev_idx] = v.astype(bf16)

    mask1 = np.where(
        np.arange(BLK)[:, None] <= np.arange(BLK)[None, :], 1.0, 0.0
    ).astype(bf16)
    mask = np.broadcast_to(
        mask1[:, None, :], (BLK, HEADS_PER_CORE, BLK)
    ).copy()

    key = tuple(nblocks)
    if key not in _GRAPH_CACHE:
        _GRAPH_CACHE[key] = _build_graph(key)
    nc = _GRAPH_CACHE[key]

    in_maps = []
    for c in range(N_CORES):
        m = {"mask": mask}
        m["kT"] = np.ascontiguousarray(kp[:, c * HEAD_DIM : (c + 1) * HEAD_DIM].T)
        m["v"] = np.ascontiguousarray(vp[:, c * HEAD_DIM : (c + 1) * HEAD_DIM])
        # [d, h, t] head-interleaved Q^T so all 4 heads ride one 3D AP
        qc = qp[:, c * HEADS_PER_CORE * HEAD_DIM : (c + 1) * HEADS_PER_CORE * HEAD_DIM]
        m["qT"] = np.ascontiguousarray(
            qc.reshape(T_pad, HEADS_PER_CORE, HEAD_DIM).transpose(2, 1, 0)
        )
        in_maps.append(m)

    trace = bool(os.environ.get("BASS_TRACE"))
    if trace:
        _install_ntff_hook()
    res = bass_utils.run_bass_kernel_spmd(
        nc, in_maps, core_ids=list(range(N_CORES)), trace=trace
    )
    if trace and res.exec_time_ns is not None:
        print(f"HW exec time: {res.exec_time_ns} ns")
        if res.instructions_and_trace is not None:
            print(f"trace: {res.instructions_and_trace[1]}")

    out = np.empty((T_host, NUM_HEADS * HEAD_DIM), np.float32)
    for c in range(N_CORES):
        r = res.results[c]
        oT = r["oT"]  # [128, H, T_pad] unnormalized
        sums = r["sums"][0]  # [H, T_pad]
        for h in range(HEADS_PER_CORE):
            gh = c * HEADS_PER_CORE + h
            o = (oT[:, h][:, dev_idx] / sums[h][dev_idx][None, :]).T
            out[:, gh * HEAD_DIM : (gh + 1) * HEAD_DIM] = o
    return out


# revision 37
# speedup vs baseline: 1.1648x; 1.0286x over previous
"""Varlen causal GQA attention on 8 TRN2 NeuronCores.

Problem: 32 q heads, 8 kv heads, head_dim 128, ragged batch (cu_seqlens),
f32. Sharded by KV-head group: core c owns kv head c and q heads
4c..4c+3 — fully data-independent across cores, no collectives.

Per core, blockwise causal attention in 128x128 blocks with all 4 q
heads fused through 3D access patterns (q stored head-interleaved
[d, h, t]), so every matmul streams exactly 4*128 = 512 columns and
each PSUM bank carries exactly one accumulation chain:
    S[k, h, q]  = (K_j)^T.T @ Q^T      ONE matmul per (q-block, k-block)
    P = exp(S * scale)                 ONE ScalarE op
    causal mask: 0/1 multiply on GpSimd (diagonal blocks only)
    O^T[h] += V_j @ P                  ONE matmul, PSUM-accumulated over j
    sums[h] += ones.T @ P              ONE M=1 matmul
Host does all transposes (Q^T/K^T in, O^T -> O out), bf16 conversion,
and the final softmax division.
"""

import math
import os
import sys

sys.path.insert(0, "/opt/trn_rl_repo")

import ml_dtypes
import numpy as np

NUM_HEADS = 32
NUM_KV_HEADS = 8
HEAD_DIM = 128
HEADS_PER_CORE = NUM_HEADS // NUM_KV_HEADS  # 4
N_CORES = 8
BLK = 128
SCALE = 1.0 / math.sqrt(HEAD_DIM)

_GRAPH_CACHE = {}


def _build_graph(seq_blocks):
    """Build the SPMD Bacc graph for padded per-seq block counts."""
    from concourse import bacc
    import concourse.mybir as mybir
    from concourse.tile import TileContext

    f32 = mybir.dt.float32
    bf16 = mybir.dt.bfloat16
    T = sum(seq_blocks) * BLK
    n_blocks_total = T // BLK
    H = HEADS_PER_CORE

    nc = bacc.Bacc("TRN2", target_bir_lowering=False, debug=False,
                   num_devices=N_CORES)

    qT_ext = nc.declare_dram_parameter("qT", [BLK, H, T], bf16, isOutput=False)
    kT_ext = nc.declare_dram_parameter("kT", [BLK, T], bf16, isOutput=False)
    v_ext = nc.declare_dram_parameter("v", [T, HEAD_DIM], bf16, isOutput=False)
    mask_ext = nc.declare_dram_parameter("mask", [BLK, H, BLK], bf16,
                                         isOutput=False)
    oT_ext = nc.declare_dram_parameter("oT", [BLK, H, T], f32, isOutput=True)
    sums_ext = nc.declare_dram_parameter("sums", [1, H, T], f32, isOutput=True)

    with TileContext(nc) as tc:
        with (
            tc.tile_pool(name="persist", bufs=1) as persist,
            tc.tile_pool(name="p", bufs=8) as p_pool,
            tc.tile_pool(name="ps_s4", bufs=2, space="PSUM") as ps_s4,
            tc.tile_pool(name="ps_o", bufs=2, space="PSUM") as ps_o,
            tc.tile_pool(name="ps_sum", bufs=2, space="PSUM") as ps_sum,
        ):
            kT_sb = persist.tile([BLK, T], bf16)
            v_sb = persist.tile([BLK, n_blocks_total, HEAD_DIM], bf16)
            mask_sb = persist.tile([BLK, H, BLK], bf16)
            qT_sb = persist.tile([BLK, H, T], bf16)
            v_re = v_ext[:].rearrange("(j p) d -> p j d", p=BLK)
            nb0 = seq_blocks[0]
            # micro-chunks for the very first blocks' data, then the rest of
            # seq 0, so the first matmul fires as early as possible
            c00 = min(4 * BLK, nb0 * BLK)
            nc.sync.dma_start(kT_sb[:, :c00], kT_ext[:, :c00])
            nc.sync.dma_start(qT_sb[:, :, :c00], qT_ext[:, :, :c00])
            nc.sync.dma_start(v_sb[:, : c00 // BLK, :], v_re[:, : c00 // BLK, :])
            if c00 < nb0 * BLK:
                nc.sync.dma_start(kT_sb[:, c00 : nb0 * BLK],
                                  kT_ext[:, c00 : nb0 * BLK])
                nc.sync.dma_start(qT_sb[:, :, c00 : nb0 * BLK],
                                  qT_ext[:, :, c00 : nb0 * BLK])
                nc.sync.dma_start(v_sb[:, c00 // BLK : nb0, :],
                                  v_re[:, c00 // BLK : nb0, :])
            nc.scalar.dma_start(mask_sb[:], mask_ext[:])
            if nb0 < n_blocks_total:
                c0 = nb0 * BLK
                nc.scalar.dma_start(kT_sb[:, c0:], kT_ext[:, c0:])
                nc.scalar.dma_start(v_sb[:, nb0:, :], v_re[:, nb0:, :])
                nc.sync.dma_start(qT_sb[:, :, c0:], qT_ext[:, :, c0:])

            ones_f = persist.tile([BLK, 1], f32)
            nc.vector.memset(ones_f[:], 1.0)
            ones_b = persist.tile([BLK, 1], bf16)
            nc.vector.tensor_copy(ones_b[:], ones_f[:])

            ot_stage = persist.tile([BLK, H, T], f32)
            sums_stage = persist.tile([1, H, T], f32)

            # flat chunk stream over (seq, q-block g, k-block j-pairs), with
            # j descending inside each group (diagonal/masked block first).
            # Each chunk = up to 2 consecutive j's whose S tiles share one
            # 2-bank PSUM tile [128, jj, h, q] so ONE exp covers both.
            chunks = []
            seq_off = 0
            for nblk in seq_blocks:
                for g in range(nblk):
                    js = list(range(g, -1, -1))
                    for i0 in range(0, len(js), 2):
                        chunks.append((seq_off, nblk, g, js[i0 : i0 + 2]))
                seq_off += nblk * BLK

            # software-pipelined ISSUE order: AV/sums trail their S/exp by
            # LAG chunks so the PE's FIFO never parks on an exp wait
            LAG = 2
            state = {}  # live group accumulators keyed by (seq_off, g)
            pending = []

            def emit_front(ch):
                seq_off, nblk, g, js = ch
                Q0 = seq_off + g * BLK
                s2 = ps_s4.tile([BLK, 2, H, BLK], f32, tag="s2", name="s2")
                for jj, j in enumerate(js):
                    kj = kT_sb[:, seq_off + j * BLK : seq_off + (j + 1) * BLK]
                    nc.tensor.matmul(
                        s2[:, jj], kj, qT_sb[:, :, Q0 : Q0 + BLK],
                        start=True, stop=True,
                    )
                p2 = p_pool.tile([BLK, 2, H, BLK], bf16, tag="p2", name="p2")
                nj = len(js)
                nc.scalar.activation(
                    p2[:, :nj], s2[:, :nj],
                    mybir.ActivationFunctionType.Exp,
                    scale=SCALE,
                )
                if js[0] == g:  # diagonal: zero the upper triangle
                    nc.gpsimd.tensor_mul(p2[:, 0], p2[:, 0], mask_sb[:])
                return p2

            def emit_back(ch, p2):
                seq_off, nblk, g, js = ch
                Q0 = seq_off + g * BLK
                key = (seq_off, g)
                if js[0] == g:
                    state[key] = (
                        ps_o.tile([BLK, H, BLK], f32, tag="ot_ps",
                                  name="oT_ps"),
                        ps_sum.tile([1, H, BLK], f32, tag="sums_ps",
                                    name="sums_ps"),
                    )
                oT_ps, sums_ps = state[key]
                for jj, j in enumerate(js):
                    vj = v_sb[:, seq_off // BLK + j, :]
                    nc.tensor.matmul(
                        oT_ps[:], vj, p2[:, jj], start=(j == g), stop=(j == 0)
                    )
                    nc.tensor.matmul(
                        sums_ps[:], ones_b[:], p2[:, jj],
                        start=(j == g), stop=(j == 0),
                    )
                if js[-1] == 0:
                    nc.vector.tensor_copy(
                        ot_stage[:, :, Q0 : Q0 + BLK], oT_ps[:]
                    )
                    nc.vector.tensor_copy(
                        sums_stage[:, :, Q0 : Q0 + BLK], sums_ps[:]
                    )
                    del state[key]
                    if g == nblk - 1:  # sequence finished: stream outputs
                        Ls = nblk * BLK
                        if seq_off + Ls == T:
                            nc.sync.dma_start(sums_ext[:], sums_stage[:])
                            for g0 in range(0, Ls, 2 * BLK):
                                W0 = min(2 * BLK, Ls - g0)
                                nc.sync.dma_start(
                                    oT_ext[:, :, seq_off + g0 : seq_off + g0 + W0],
                                    ot_stage[:, :, seq_off + g0 : seq_off + g0 + W0],
                                )
                        else:
                            nc.sync.dma_start(
                                oT_ext[:, :, seq_off : seq_off + Ls],
                                ot_stage[:, :, seq_off : seq_off + Ls],
                            )

            for ch in chunks:
                p2 = emit_front(ch)
                pending.append((ch, p2))
                if len(pending) > LAG:
                    emit_back(*pending.pop(0))
            for ch, p2 in pending:
                emit_back(ch, p2)

    nc.finalize()
    return nc


def _install_ntff_hook():
    """Shim antenv.axon_hooks (absent in this container) so trace=True can
    reach the terminal's NRT profiler via libaxon_pjrt.so ctypes."""
    import types

    if "antenv.axon_hooks" in sys.modules:
        return
    import antenv
    from concourse import bass_utils

    mod = types.ModuleType("antenv.axon_hooks")
    state = {"hook": None}
    mod.set_axon_ntff_profile_hook = lambda h: state.__setitem__("hook", h)
    mod.get_axon_ntff_profile_hook = lambda: state["hook"]
    sys.modules["antenv.axon_hooks"] = mod
    antenv.axon_hooks = mod
    bass_utils.upload_artifacts = lambda tmpdir: tmpdir  # zero-egress container
    try:
        if "/root/.axon_site" not in sys.path:
            sys.path.insert(0, "/root/.axon_site")
        from trn_agent_boot.trn_boot import _ntff_profile_via_ctypes

        mod.set_axon_ntff_profile_hook(
            _ntff_profile_via_ctypes("/opt/axon/libaxon_pjrt.so")
        )
    except Exception:
        pass


def kernel(q, k, v, cu_seqlens, max_seqlen):
    from concourse import bass_utils

    q = np.asarray(q, dtype=np.float32)
    k = np.asarray(k, dtype=np.float32)
    v = np.asarray(v, dtype=np.float32)
    cu = np.asarray(cu_seqlens, dtype=np.int64)
    T_host = q.shape[0]
    lengths = np.diff(cu).astype(np.int64)
    all_nblocks = [int((L + BLK - 1) // BLK) for L in lengths]
    T_pad = sum(all_nblocks) * BLK

    # process sequences longest-first: big seq warms the pipeline while the
    # rest of the data streams in, and the tail drains a small seq
    order = sorted(range(len(lengths)), key=lambda s: -all_nblocks[s])
    nblocks = [all_nblocks[s] for s in order]

    # host -> padded device token index map (valid tokens only)
    dev_idx = np.zeros(T_host, dtype=np.int64)
    pad_off = 0
    for s in order:
        L = int(lengths[s])
        dev_idx[cu[s] : cu[s] + L] = pad_off + np.arange(L)
        pad_off += all_nblocks[s] * BLK

    bf16 = ml_dtypes.bfloat16
    qp = np.zeros((T_pad, NUM_HEADS * HEAD_DIM), bf16)
    kp = np.zeros((T_pad, NUM_KV_HEADS * HEAD_DIM), bf16)
    vp = np.zeros((T_pad, NUM_KV_HEADS * HEAD_DIM), bf16)
    qp[dev_idx] = q.astype(bf16)
    kp[dev_idx] = k.astype(bf16)
    vp[dev_idx] = v.astype(bf16)

    mask1 = np.where(
        np.arange(BLK)[:, None] <= np.arange(BLK)[None, :], 1.0, 0.0
    ).astype(bf16)
    mask = np.broadcast_to(
        mask1[:, None, :], (BLK, HEADS_PER_CORE, BLK)
    ).copy()

    key = tuple(nblocks)
    if key not in _GRAPH_CACHE:
        _GRAPH_CACHE[key] = _build_graph(key)
    nc = _GRAPH_CACHE[key]

    in_maps = []
    for c in range(N_CORES):
        m = {"mask": mask}
        m["kT"] = np.ascontiguousarray(kp[:, c * HEAD_DIM : (c + 1) * HEAD_DIM].T)
        m["v"] = np.ascontiguousarray(vp[:, c * HEAD_DIM : (c + 1) * HEAD_DIM])
        # [d, h, t] head-interleaved Q^T so all 4 heads ride one 3D AP
        qc = qp[:, c * HEADS_PER_CORE * HEAD_DIM : (c + 1) * HEADS_PER_CORE * HEAD_DIM]
        m["qT"] = np.ascontiguousarray(
            qc.reshape(T_pad, HEADS_PER_CORE, HEAD_DIM).transpose(2, 1, 0)
        )
        in_maps.append(m)

    trace = bool(os.environ.get("BASS_TRACE"))
    if trace:
        _install_ntff_hook()
    res = bass_utils.run_bass_kernel_spmd(
        nc, in_maps, core_ids=list(range(N_CORES)), trace=trace
    )
    if trace and res.exec_time_ns is not None:
        print(f"HW exec time: {res.exec_time_ns} ns")
        if res.instructions_and_trace is not None:
            print(f"trace: {res.instructions_and_trace[1]}")

    out = np.empty((T_host, NUM_HEADS * HEAD_DIM), np.float32)
    for c in range(N_CORES):
        r = res.results[c]
        oT = r["oT"]  # [128, H, T_pad] unnormalized
        sums = r["sums"][0]  # [H, T_pad]
        for h in range(HEADS_PER_CORE):
            gh = c * HEADS_PER_CORE + h
            o = (oT[:, h][:, dev_idx] / sums[h][dev_idx][None, :]).T
            out[:, gh * HEAD_DIM : (gh + 1) * HEAD_DIM] = o
    return out


# revision 41
# speedup vs baseline: 1.1759x; 1.0096x over previous
"""Varlen causal GQA attention on 8 TRN2 NeuronCores.

Problem: 32 q heads, 8 kv heads, head_dim 128, ragged batch (cu_seqlens),
f32. Sharded by KV-head group: core c owns kv head c and q heads
4c..4c+3 — fully data-independent across cores, no collectives.

Per core, blockwise causal attention in 128x128 blocks with all 4 q
heads fused through 3D access patterns (q stored head-interleaved
[d, h, t]), so every matmul streams exactly 4*128 = 512 columns and
each PSUM bank carries exactly one accumulation chain:
    S[k, h, q]  = (K_j)^T.T @ Q^T      ONE matmul per (q-block, k-block)
    P = exp(S * scale)                 ONE ScalarE op
    causal mask: 0/1 multiply on GpSimd (diagonal blocks only)
    O^T[h] += V_j @ P                  ONE matmul, PSUM-accumulated over j
    sums[h] += ones.T @ P              ONE M=1 matmul
Host does all transposes (Q^T/K^T in, O^T -> O out), bf16 conversion,
and the final softmax division.
"""

import math
import os
import sys

sys.path.insert(0, "/opt/trn_rl_repo")

import ml_dtypes
import numpy as np

NUM_HEADS = 32
NUM_KV_HEADS = 8
HEAD_DIM = 128
HEADS_PER_CORE = NUM_HEADS // NUM_KV_HEADS  # 4
N_CORES = 8
BLK = 128
SCALE = 1.0 / math.sqrt(HEAD_DIM)

_GRAPH_CACHE = {}


def _build_graph(seq_blocks):
    """Build the SPMD Bacc graph for padded per-seq block counts."""
    from concourse import bacc
    import concourse.mybir as mybir
    from concourse.tile import TileContext

    f32 = mybir.dt.float32
    bf16 = mybir.dt.bfloat16
    T = sum(seq_blocks) * BLK
    n_blocks_total = T // BLK
    H = HEADS_PER_CORE

    nc = bacc.Bacc("TRN2", target_bir_lowering=False, debug=False,
                   num_devices=N_CORES)

    qT_ext = nc.declare_dram_parameter("qT", [BLK, H, T], bf16, isOutput=False)
    kT_ext = nc.declare_dram_parameter("kT", [BLK, T], bf16, isOutput=False)
    v_ext = nc.declare_dram_parameter("v", [T, HEAD_DIM], bf16, isOutput=False)
    mask_ext = nc.declare_dram_parameter("mask", [BLK, H, BLK], bf16,
                                         isOutput=False)
    oT_ext = nc.declare_dram_parameter("oT", [BLK, H, T], f32, isOutput=True)
    sums_ext = nc.declare_dram_parameter("sums", [1, H, T], f32, isOutput=True)

    with TileContext(nc) as tc:
        with (
            tc.tile_pool(name="persist", bufs=1) as persist,
            tc.tile_pool(name="p", bufs=8) as p_pool,
            tc.tile_pool(name="ps_s4", bufs=2, space="PSUM") as ps_s4,
            tc.tile_pool(name="ps_o", bufs=2, space="PSUM") as ps_o,
            tc.tile_pool(name="ps_sum", bufs=2, space="PSUM") as ps_sum,
        ):
            kT_sb = persist.tile([BLK, T], bf16)
            v_sb = persist.tile([BLK, n_blocks_total, HEAD_DIM], bf16)
            mask_sb = persist.tile([BLK, H, BLK], bf16)
            qT_sb = persist.tile([BLK, H, T], bf16)
            v_re = v_ext[:].rearrange("(j p) d -> p j d", p=BLK)
            nb0 = seq_blocks[0]
            # dependencies are tile-granular: duplicate the first few blocks
            # into separate small tiles so the opening matmuls depend only on
            # tiny DMAs, not on the full-tensor loads running behind them
            nh = min(4, nb0)
            c00 = nh * BLK
            kT_head = persist.tile([BLK, c00], bf16)
            qT_head = persist.tile([BLK, H, c00], bf16)
            v_head = persist.tile([BLK, nh, HEAD_DIM], bf16)
            nc.sync.dma_start(qT_head[:], qT_ext[:, :, :c00])
            nc.sync.dma_start(kT_head[:], kT_ext[:, :c00])
            nc.sync.dma_start(v_head[:], v_re[:, :nh, :])
            nc.sync.dma_start(qT_sb[:, :, : nb0 * BLK],
                              qT_ext[:, :, : nb0 * BLK])
            nc.sync.dma_start(kT_sb[:, : nb0 * BLK], kT_ext[:, : nb0 * BLK])
            nc.sync.dma_start(v_sb[:, :nb0, :], v_re[:, :nb0, :])
            nc.scalar.dma_start(mask_sb[:], mask_ext[:])
            if nb0 < n_blocks_total:
                c0 = nb0 * BLK
                nc.scalar.dma_start(kT_sb[:, c0:], kT_ext[:, c0:])
                nc.scalar.dma_start(v_sb[:, nb0:, :], v_re[:, nb0:, :])
                nc.sync.dma_start(qT_sb[:, :, c0:], qT_ext[:, :, c0:])

            ones_f = persist.tile([BLK, 1], f32)
            nc.vector.memset(ones_f[:], 1.0)
            ones_b = persist.tile([BLK, 1], bf16)
            nc.vector.tensor_copy(ones_b[:], ones_f[:])

            ot_stage = persist.tile([BLK, H, T], f32)
            sums_stage = persist.tile([1, H, T], f32)

            # flat chunk stream over (seq, q-block g, k-block j-pairs), with
            # j descending inside each group (diagonal/masked block first).
            # Each chunk = up to 2 consecutive j's whose S tiles share one
            # 2-bank PSUM tile [128, jj, h, q] so ONE exp covers both.
            chunks = []
            seq_off = 0
            for nblk in seq_blocks:
                for g in range(nblk):
                    js = list(range(g, -1, -1))
                    for i0 in range(0, len(js), 2):
                        chunks.append((seq_off, nblk, g, js[i0 : i0 + 2]))
                seq_off += nblk * BLK

            # software-pipelined ISSUE order: AV/sums trail their S/exp by
            # LAG chunks so the PE's FIFO never parks on an exp wait
            LAG = 2
            state = {}  # live group accumulators keyed by (seq_off, g)
            pending = []

            def emit_front(ch):
                seq_off, nblk, g, js = ch
                Q0 = seq_off + g * BLK
                early = seq_off == 0 and g < nh
                s2 = ps_s4.tile([BLK, 2, H, BLK], f32, tag="s2", name="s2")
                for jj, j in enumerate(js):
                    if early:
                        kj = kT_head[:, j * BLK : (j + 1) * BLK]
                        qg = qT_head[:, :, g * BLK : (g + 1) * BLK]
                    else:
                        kj = kT_sb[:, seq_off + j * BLK : seq_off + (j + 1) * BLK]
                        qg = qT_sb[:, :, Q0 : Q0 + BLK]
                    nc.tensor.matmul(
                        s2[:, jj], kj, qg,
                        start=True, stop=True,
                    )
                p2 = p_pool.tile([BLK, 2, H, BLK], bf16, tag="p2", name="p2")
                nj = len(js)
                nc.scalar.activation(
                    p2[:, :nj], s2[:, :nj],
                    mybir.ActivationFunctionType.Exp,
                    scale=SCALE,
                )
                if js[0] == g:  # diagonal: zero the upper triangle
                    nc.gpsimd.tensor_mul(p2[:, 0], p2[:, 0], mask_sb[:])
                return p2

            def emit_back(ch, p2):
                seq_off, nblk, g, js = ch
                Q0 = seq_off + g * BLK
                key = (seq_off, g)
                if js[0] == g:
                    state[key] = (
                        ps_o.tile([BLK, H, BLK], f32, tag="ot_ps",
                                  name="oT_ps"),
                        ps_sum.tile([1, H, BLK], f32, tag="sums_ps",
                                    name="sums_ps"),
                    )
                oT_ps, sums_ps = state[key]
                for jj, j in enumerate(js):
                    if seq_off == 0 and g < nh:
                        vj = v_head[:, j, :]
                    else:
                        vj = v_sb[:, seq_off // BLK + j, :]
                    nc.tensor.matmul(
                        oT_ps[:], vj, p2[:, jj], start=(j == g), stop=(j == 0)
                    )
                    nc.tensor.matmul(
                        sums_ps[:], ones_b[:], p2[:, jj],
                        start=(j == g), stop=(j == 0),
                    )
                if js[-1] == 0:
                    nc.vector.tensor_copy(
                        ot_stage[:, :, Q0 : Q0 + BLK], oT_ps[:]
                    )
                    nc.vector.tensor_copy(
                        sums_stage[:, :, Q0 : Q0 + BLK], sums_ps[:]
                    )
                    del state[key]
                    if g == nblk - 1:  # sequence finished: stream outputs
                        Ls = nblk * BLK
                        if seq_off + Ls == T:
                            nc.sync.dma_start(sums_ext[:], sums_stage[:])
                            for g0 in range(0, Ls, BLK):
                                nc.sync.dma_start(
                                    oT_ext[:, :, seq_off + g0 : seq_off + g0 + BLK],
                                    ot_stage[:, :, seq_off + g0 : seq_off + g0 + BLK],
                                )
                        else:
                            nc.sync.dma_start(
                                oT_ext[:, :, seq_off : seq_off + Ls],
                                ot_stage[:, :, seq_off : seq_off + Ls],
                            )

            for ch in chunks:
                p2 = emit_front(ch)
                pending.append((ch, p2))
                if len(pending) > LAG:
                    emit_back(*pending.pop(0))
            for ch, p2 in pending:
                emit_back(ch, p2)

    nc.finalize()
    return nc


def _install_ntff_hook():
    """Shim antenv.axon_hooks (absent in this container) so trace=True can
    reach the terminal's NRT profiler via libaxon_pjrt.so ctypes."""
    import types

    if "antenv.axon_hooks" in sys.modules:
        return
    import antenv
    from concourse import bass_utils

    mod = types.ModuleType("antenv.axon_hooks")
    state = {"hook": None}
    mod.set_axon_ntff_profile_hook = lambda h: state.__setitem__("hook", h)
    mod.get_axon_ntff_profile_hook = lambda: state["hook"]
    sys.modules["antenv.axon_hooks"] = mod
    antenv.axon_hooks = mod
    bass_utils.upload_artifacts = lambda tmpdir: tmpdir  # zero-egress container
    try:
        if "/root/.axon_site" not in sys.path:
            sys.path.insert(0, "/root/.axon_site")
        from trn_agent_boot.trn_boot import _ntff_profile_via_ctypes

        mod.set_axon_ntff_profile_hook(
            _ntff_profile_via_ctypes("/opt/axon/libaxon_pjrt.so")
        )
    except Exception:
        pass


def kernel(q, k, v, cu_seqlens, max_seqlen):
    from concourse import bass_utils

    q = np.asarray(q, dtype=np.float32)
    k = np.asarray(k, dtype=np.float32)
    v = np.asarray(v, dtype=np.float32)
    cu = np.asarray(cu_seqlens, dtype=np.int64)
    T_host = q.shape[0]
    lengths = np.diff(cu).astype(np.int64)
    all_nblocks = [int((L + BLK - 1) // BLK) for L in lengths]
    T_pad = sum(all_nblocks) * BLK

    # process sequences longest-first: big seq warms the pipeline while the
    # rest of the data streams in, and the tail drains a small seq
    order = sorted(range(len(lengths)), key=lambda s: -all_nblocks[s])
    nblocks = [all_nblocks[s] for s in order]

    # host -> padded device token index map (valid tokens only)
    dev_idx = np.zeros(T_host, dtype=np.int64)
    pad_off = 0
    for s in order:
        L = int(lengths[s])
        dev_idx[cu[s] : cu[s] + L] = pad_off + np.arange(L)
        pad_off += all_nblocks[s] * BLK

    bf16 = ml_dtypes.bfloat16
    qp = np.zeros((T_pad, NUM_HEADS * HEAD_DIM), bf16)
    kp = np.zeros((T_pad, NUM_KV_HEADS * HEAD_DIM), bf16)
    vp = np.zeros((T_pad, NUM_KV_HEADS * HEAD_DIM), bf16)
    qp[dev_idx] = q.astype(bf16)
    kp[dev_idx] = k.astype(bf16)
    vp[dev_idx] = v.astype(bf16)

    mask1 = np.where(
        np.arange(BLK)[:, None] <= np.arange(BLK)[None, :], 1.0, 0.0
    ).astype(bf16)
    mask = np.broadcast_to(
        mask1[:, None, :], (BLK, HEADS_PER_CORE, BLK)
    ).copy()

    key = tuple(nblocks)
    if key not in _GRAPH_CACHE:
        _GRAPH_CACHE[key] = _build_graph(key)
    nc = _GRAPH_CACHE[key]

    in_maps = []
    for c in range(N_CORES):
        m = {"mask": mask}
        m["kT"] = np.ascontiguousarray(kp[:, c * HEAD_DIM : (c + 1) * HEAD_DIM].T)
        m["v"] = np.ascontiguousarray(vp[:, c * HEAD_DIM : (c + 1) * HEAD_DIM])
        # [d, h, t] head-interleaved Q^T so all 4 heads ride one 3D AP
        qc = qp[:, c * HEADS_PER_CORE * HEAD_DIM : (c + 1) * HEADS_PER_CORE * HEAD_DIM]
        m["qT"] = np.ascontiguousarray(
            qc.reshape(T_pad, HEADS_PER_CORE, HEAD_DIM).transpose(2, 1, 0)
        )
        in_maps.append(m)

    trace = bool(os.environ.get("BASS_TRACE"))
    if trace:
        _install_ntff_hook()
    res = bass_utils.run_bass_kernel_spmd(
        nc, in_maps, core_ids=list(range(N_CORES)), trace=trace
    )
    if trace and res.exec_time_ns is not None:
        print(f"HW exec time: {res.exec_time_ns} ns")
        if res.instructions_and_trace is not None:
            print(f"trace: {res.instructions_and_trace[1]}")

    out = np.empty((T_host, NUM_HEADS * HEAD_DIM), np.float32)
    for c in range(N_CORES):
        r = res.results[c]
        oT = r["oT"]  # [128, H, T_pad] unnormalized
        sums = r["sums"][0]  # [H, T_pad]
        for h in range(HEADS_PER_CORE):
            gh = c * HEADS_PER_CORE + h
            o = (oT[:, h][:, dev_idx] / sums[h][dev_idx][None, :]).T
            out[:, gh * HEAD_DIM : (gh + 1) * HEAD_DIM] = o
    return out


# revision 48
# speedup vs baseline: 1.2432x; 1.0572x over previous
"""Varlen causal GQA attention on 8 TRN2 NeuronCores.

Problem: 32 q heads, 8 kv heads, head_dim 128, ragged batch (cu_seqlens),
f32. Sharded by KV-head group: core c owns kv head c and q heads
4c..4c+3 — fully data-independent across cores, no collectives.

Per core, blockwise causal attention in 128x128 blocks with all 4 q
heads fused through 3D access patterns (q stored head-interleaved
[d, h, t]), so every matmul streams exactly 4*128 = 512 columns and
each PSUM bank carries exactly one accumulation chain:
    S[k, h, q]  = (K_j)^T.T @ Q^T      ONE matmul per (q-block, k-block)
    P = exp(S * scale)                 ONE ScalarE op
    causal mask: 0/1 multiply on GpSimd (diagonal blocks only)
    O^T[h] += V_j @ P                  ONE matmul, PSUM-accumulated over j
    sums[h] += ones.T @ P              ONE M=1 matmul
Host does all transposes (Q^T/K^T in, O^T -> O out), bf16 conversion,
and the final softmax division.
"""

import math
import os
import sys

sys.path.insert(0, "/opt/trn_rl_repo")

import ml_dtypes
import numpy as np

NUM_HEADS = 32
NUM_KV_HEADS = 8
HEAD_DIM = 128
HEADS_PER_CORE = NUM_HEADS // NUM_KV_HEADS  # 4
N_CORES = 8
BLK = 128
SCALE = 1.0 / math.sqrt(HEAD_DIM)

_GRAPH_CACHE = {}


def _build_graph(seq_blocks):
    """Build the SPMD Bacc graph for padded per-seq block counts."""
    from concourse import bacc
    import concourse.mybir as mybir
    from concourse.tile import TileContext

    f32 = mybir.dt.float32
    bf16 = mybir.dt.bfloat16
    T = sum(seq_blocks) * BLK
    n_blocks_total = T // BLK
    H = HEADS_PER_CORE

    nc = bacc.Bacc("TRN2", target_bir_lowering=False, debug=False,
                   num_devices=N_CORES)

    qT_ext = nc.declare_dram_parameter("qT", [BLK, H, T], bf16, isOutput=False)
    kT_ext = nc.declare_dram_parameter("kT", [BLK, T], bf16, isOutput=False)
    v_ext = nc.declare_dram_parameter("v", [T, HEAD_DIM], bf16, isOutput=False)
    mask_ext = nc.declare_dram_parameter("mask", [BLK, H, BLK], bf16,
                                         isOutput=False)
    oT_ext = nc.declare_dram_parameter("oT", [BLK, H, T], f32, isOutput=True)
    sums_ext = nc.declare_dram_parameter("sums", [1, H, T], f32, isOutput=True)

    with TileContext(nc) as tc:
        with (
            tc.tile_pool(name="persist", bufs=1) as persist,
            tc.tile_pool(name="p", bufs=8) as p_pool,
            tc.tile_pool(name="ps_s4", bufs=2, space="PSUM") as ps_s4,
            tc.tile_pool(name="ps_o", bufs=2, space="PSUM") as ps_o,
            tc.tile_pool(name="ps_sum", bufs=2, space="PSUM") as ps_sum,
        ):
            kT_sb = persist.tile([BLK, T], bf16)
            v_sb = persist.tile([BLK, n_blocks_total, HEAD_DIM], bf16)
            mask_sb = persist.tile([BLK, H, BLK], bf16)
            qT_sb = persist.tile([BLK, H, T], bf16)
            v_re = v_ext[:].rearrange("(j p) d -> p j d", p=BLK)
            nb0 = seq_blocks[0]
            # dependencies are tile-granular: duplicate the first few blocks
            # into separate small tiles so the opening matmuls depend only on
            # tiny DMAs, not on the full-tensor loads running behind them
            nh = min(4, nb0)
            c00 = nh * BLK
            kT_head = persist.tile([BLK, c00], bf16)
            qT_head = persist.tile([BLK, H, c00], bf16)
            v_head = persist.tile([BLK, nh, HEAD_DIM], bf16)
            nc.scalar.dma_start(qT_head[:], qT_ext[:, :, :c00])
            nc.scalar.dma_start(kT_head[:], kT_ext[:, :c00])
            nc.scalar.dma_start(v_head[:], v_re[:, :nh, :])
            nc.sync.dma_start(qT_sb[:, :, : nb0 * BLK],
                              qT_ext[:, :, : nb0 * BLK])
            nc.sync.dma_start(kT_sb[:, : nb0 * BLK], kT_ext[:, : nb0 * BLK])
            nc.sync.dma_start(v_sb[:, :nb0, :], v_re[:, :nb0, :])
            nc.scalar.dma_start(mask_sb[:], mask_ext[:])
            if nb0 < n_blocks_total:
                c0 = nb0 * BLK
                nc.scalar.dma_start(kT_sb[:, c0:], kT_ext[:, c0:])
                nc.scalar.dma_start(v_sb[:, nb0:, :], v_re[:, nb0:, :])
                nc.sync.dma_start(qT_sb[:, :, c0:], qT_ext[:, :, c0:])

            ones_f = persist.tile([BLK, BLK], f32)
            nc.vector.memset(ones_f[:], 1.0)
            # full [128,128] ones stationary: sums matmul runs M=128 so the
            # PE array never reconfigures col groups between AV and sums
            ones_b = persist.tile([BLK, BLK], bf16)
            nc.vector.tensor_copy(ones_b[:], ones_f[:])

            ot_stage = persist.tile([BLK, H, T], f32)
            sums_stage = persist.tile([1, H, T], f32)

            # flat chunk stream over (seq, q-block g, k-block j-pairs), with
            # j descending inside each group (diagonal/masked block first).
            # Each chunk = up to 2 consecutive j's whose S tiles share one
            # 2-bank PSUM tile [128, jj, h, q] so ONE exp covers both.
            chunks = []
            seq_off = 0
            for nblk in seq_blocks:
                for g in range(nblk):
                    js = list(range(g, -1, -1))
                    for i0 in range(0, len(js), 2):
                        chunks.append((seq_off, nblk, g, js[i0 : i0 + 2]))
                seq_off += nblk * BLK

            # software-pipelined ISSUE order: AV/sums trail their S/exp by
            # LAG chunks so the PE's FIFO never parks on an exp wait
            LAG = 2
            state = {}  # live group accumulators keyed by (seq_off, g)
            pending = []

            def emit_front(ch):
                seq_off, nblk, g, js = ch
                Q0 = seq_off + g * BLK
                early = seq_off == 0 and g < nh
                s2 = ps_s4.tile([BLK, 2, H, BLK], f32, tag="s2", name="s2")
                for jj, j in enumerate(js):
                    if early:
                        kj = kT_head[:, j * BLK : (j + 1) * BLK]
                        qg = qT_head[:, :, g * BLK : (g + 1) * BLK]
                    else:
                        kj = kT_sb[:, seq_off + j * BLK : seq_off + (j + 1) * BLK]
                        qg = qT_sb[:, :, Q0 : Q0 + BLK]
                    nc.tensor.matmul(
                        s2[:, jj], kj, qg,
                        start=True, stop=True,
                    )
                p2 = p_pool.tile([BLK, 2, H, BLK], bf16, tag="p2", name="p2")
                nj = len(js)
                nc.scalar.activation(
                    p2[:, :nj], s2[:, :nj],
                    mybir.ActivationFunctionType.Exp,
                    scale=SCALE,
                )
                if js[0] == g:  # diagonal: zero the upper triangle
                    nc.gpsimd.tensor_mul(p2[:, 0], p2[:, 0], mask_sb[:])
                return p2

            def emit_back(ch, p2):
                seq_off, nblk, g, js = ch
                Q0 = seq_off + g * BLK
                key = (seq_off, g)
                if js[0] == g:
                    state[key] = (
                        ps_o.tile([BLK, H, BLK], f32, tag="ot_ps",
                                  name="oT_ps"),
                        ps_sum.tile([BLK, H, BLK], f32, tag="sums_ps",
                                    name="sums_ps"),
                    )
                oT_ps, sums_ps = state[key]
                for jj, j in enumerate(js):
                    if seq_off == 0 and g < nh:
                        vj = v_head[:, j, :]
                    else:
                        vj = v_sb[:, seq_off // BLK + j, :]
                    nc.tensor.matmul(
                        oT_ps[:], vj, p2[:, jj], start=(j == g), stop=(j == 0)
                    )
                for jj, j in enumerate(js):
                    nc.tensor.matmul(
                        sums_ps[:], ones_b[:], p2[:, jj],
                        start=(j == g), stop=(j == 0),
                    )
                if js[-1] == 0:
                    nc.vector.tensor_copy(
                        ot_stage[:, :, Q0 : Q0 + BLK], oT_ps[:]
                    )
                    nc.vector.tensor_copy(
                        sums_stage[:, :, Q0 : Q0 + BLK], sums_ps[0:1]
                    )
                    del state[key]
                    if g == nblk - 1:  # sequence finished: stream outputs
                        Ls = nblk * BLK
                        if seq_off + Ls == T:
                            nc.sync.dma_start(sums_ext[:], sums_stage[:])
                            for g0 in range(0, Ls, BLK):
                                nc.sync.dma_start(
                                    oT_ext[:, :, seq_off + g0 : seq_off + g0 + BLK],
                                    ot_stage[:, :, seq_off + g0 : seq_off + g0 + BLK],
                                )
                        else:
                            nc.sync.dma_start(
                                oT_ext[:, :, seq_off : seq_off + Ls],
                                ot_stage[:, :, seq_off : seq_off + Ls],
                            )

            for ch in chunks:
                p2 = emit_front(ch)
                pending.append((ch, p2))
                if len(pending) > LAG:
                    emit_back(*pending.pop(0))
            for ch, p2 in pending:
                emit_back(ch, p2)

    nc.finalize()
    return nc


def _install_ntff_hook():
    """Shim antenv.axon_hooks (absent in this container) so trace=True can
    reach the terminal's NRT profiler via libaxon_pjrt.so ctypes."""
    import types

    if "antenv.axon_hooks" in sys.modules:
        return
    import antenv
    from concourse import bass_utils

    mod = types.ModuleType("antenv.axon_hooks")
    state = {"hook": None}
    mod.set_axon_ntff_profile_hook = lambda h: state.__setitem__("hook", h)
    mod.get_axon_ntff_profile_hook = lambda: state["hook"]
    sys.modules["antenv.axon_hooks"] = mod
    antenv.axon_hooks = mod
    bass_utils.upload_artifacts = lambda tmpdir: tmpdir  # zero-egress container
    try:
        if "/root/.axon_site" not in sys.path:
            sys.path.insert(0, "/root/.axon_site")
        from trn_agent_boot.trn_boot import _ntff_profile_via_ctypes

        mod.set_axon_ntff_profile_hook(
            _ntff_profile_via_ctypes("/opt/axon/libaxon_pjrt.so")
        )
    except Exception:
        pass


def kernel(q, k, v, cu_seqlens, max_seqlen):
    from concourse import bass_utils

    q = np.asarray(q, dtype=np.float32)
    k = np.asarray(k, dtype=np.float32)
    v = np.asarray(v, dtype=np.float32)
    cu = np.asarray(cu_seqlens, dtype=np.int64)
    T_host = q.shape[0]
    lengths = np.diff(cu).astype(np.int64)
    all_nblocks = [int((L + BLK - 1) // BLK) for L in lengths]
    T_pad = sum(all_nblocks) * BLK

    # process sequences longest-first: big seq warms the pipeline while the
    # rest of the data streams in, and the tail drains a small seq
    order = sorted(range(len(lengths)), key=lambda s: -all_nblocks[s])
    nblocks = [all_nblocks[s] for s in order]

    # host -> padded device token index map (valid tokens only)
    dev_idx = np.zeros(T_host, dtype=np.int64)
    pad_off = 0
    for s in order:
        L = int(lengths[s])
        dev_idx[cu[s] : cu[s] + L] = pad_off + np.arange(L)
        pad_off += all_nblocks[s] * BLK

    bf16 = ml_dtypes.bfloat16
    qp = np.zeros((T_pad, NUM_HEADS * HEAD_DIM), bf16)
    kp = np.zeros((T_pad, NUM_KV_HEADS * HEAD_DIM), bf16)
    vp = np.zeros((T_pad, NUM_KV_HEADS * HEAD_DIM), bf16)
    qp[dev_idx] = q.astype(bf16)
    kp[dev_idx] = k.astype(bf16)
    vp[dev_idx] = v.astype(bf16)

    mask1 = np.where(
        np.arange(BLK)[:, None] <= np.arange(BLK)[None, :], 1.0, 0.0
    ).astype(bf16)
    mask = np.broadcast_to(
        mask1[:, None, :], (BLK, HEADS_PER_CORE, BLK)
    ).copy()

    key = tuple(nblocks)
    if key not in _GRAPH_CACHE:
        _GRAPH_CACHE[key] = _build_graph(key)
    nc = _GRAPH_CACHE[key]

    in_maps = []
    for c in range(N_CORES):
        m = {"mask": mask}
        m["kT"] = np.ascontiguousarray(kp[:, c * HEAD_DIM : (c + 1) * HEAD_DIM].T)
        m["v"] = np.ascontiguousarray(vp[:, c * HEAD_DIM : (c + 1) * HEAD_DIM])
        # [d, h, t] head-interleaved Q^T so all 4 heads ride one 3D AP
        qc = qp[:, c * HEADS_PER_CORE * HEAD_DIM : (c + 1) * HEADS_PER_CORE * HEAD_DIM]
        m["qT"] = np.ascontiguousarray(
            qc.reshape(T_pad, HEADS_PER_CORE, HEAD_DIM).transpose(2, 1, 0)
        )
        in_maps.append(m)

    trace = bool(os.environ.get("BASS_TRACE"))
    if trace:
        _install_ntff_hook()
    res = bass_utils.run_bass_kernel_spmd(
        nc, in_maps, core_ids=list(range(N_CORES)), trace=trace
    )
    if trace and res.exec_time_ns is not None:
        print(f"HW exec time: {res.exec_time_ns} ns")
        if res.instructions_and_trace is not None:
            print(f"trace: {res.instructions_and_trace[1]}")

    out = np.empty((T_host, NUM_HEADS * HEAD_DIM), np.float32)
    for c in range(N_CORES):
        r = res.results[c]
        oT = r["oT"]  # [128, H, T_pad] unnormalized
        sums = r["sums"][0]  # [H, T_pad]
        for h in range(HEADS_PER_CORE):
            gh = c * HEADS_PER_CORE + h
            o = (oT[:, h][:, dev_idx] / sums[h][dev_idx][None, :]).T
            out[:, gh * HEAD_DIM : (gh + 1) * HEAD_DIM] = o
    return out


# revision 49
# speedup vs baseline: 1.3012x; 1.0467x over previous
"""Varlen causal GQA attention on 8 TRN2 NeuronCores.

Problem: 32 q heads, 8 kv heads, head_dim 128, ragged batch (cu_seqlens),
f32. Sharded by KV-head group: core c owns kv head c and q heads
4c..4c+3 — fully data-independent across cores, no collectives.

Per core, blockwise causal attention in 128x128 blocks with all 4 q
heads fused through 3D access patterns (q stored head-interleaved
[d, h, t]), so every matmul streams exactly 4*128 = 512 columns and
each PSUM bank carries exactly one accumulation chain:
    S[k, h, q]  = (K_j)^T.T @ Q^T      ONE matmul per (q-block, k-block)
    P = exp(S * scale)                 ONE ScalarE op
    causal mask: 0/1 multiply on GpSimd (diagonal blocks only)
    O^T[h] += V_j @ P                  ONE matmul, PSUM-accumulated over j
    sums[h] += ones.T @ P              ONE M=1 matmul
Host does all transposes (Q^T/K^T in, O^T -> O out), bf16 conversion,
and the final softmax division.
"""

import math
import os
import sys

sys.path.insert(0, "/opt/trn_rl_repo")

import ml_dtypes
import numpy as np

NUM_HEADS = 32
NUM_KV_HEADS = 8
HEAD_DIM = 128
HEADS_PER_CORE = NUM_HEADS // NUM_KV_HEADS  # 4
N_CORES = 8
BLK = 128
SCALE = 1.0 / math.sqrt(HEAD_DIM)

_GRAPH_CACHE = {}


def _build_graph(seq_blocks):
    """Build the SPMD Bacc graph for padded per-seq block counts."""
    from concourse import bacc
    import concourse.mybir as mybir
    from concourse.tile import TileContext

    f32 = mybir.dt.float32
    bf16 = mybir.dt.bfloat16
    T = sum(seq_blocks) * BLK
    n_blocks_total = T // BLK
    H = HEADS_PER_CORE

    nc = bacc.Bacc("TRN2", target_bir_lowering=False, debug=False,
                   num_devices=N_CORES)

    qT_ext = nc.declare_dram_parameter("qT", [BLK, H, T], bf16, isOutput=False)
    kT_ext = nc.declare_dram_parameter("kT", [BLK, T], bf16, isOutput=False)
    v_ext = nc.declare_dram_parameter("v", [T, HEAD_DIM], bf16, isOutput=False)
    mask_ext = nc.declare_dram_parameter("mask", [BLK, H, BLK], bf16,
                                         isOutput=False)
    oT_ext = nc.declare_dram_parameter("oT", [BLK, H, T], f32, isOutput=True)
    sums_ext = nc.declare_dram_parameter("sums", [1, H, T], f32, isOutput=True)

    with TileContext(nc) as tc:
        with (
            tc.tile_pool(name="persist", bufs=1) as persist,
            tc.tile_pool(name="p", bufs=8) as p_pool,
            tc.tile_pool(name="ps_s4", bufs=2, space="PSUM") as ps_s4,
            tc.tile_pool(name="ps_o", bufs=2, space="PSUM") as ps_o,
            tc.tile_pool(name="ps_sum", bufs=2, space="PSUM") as ps_sum,
        ):
            kT_sb = persist.tile([BLK, T], bf16)
            v_sb = persist.tile([BLK, n_blocks_total, HEAD_DIM], bf16)
            mask_sb = persist.tile([BLK, H, BLK], bf16)
            qT_sb = persist.tile([BLK, H, T], bf16)
            v_re = v_ext[:].rearrange("(j p) d -> p j d", p=BLK)
            nb0 = seq_blocks[0]
            # dependencies are tile-granular: duplicate the first few blocks
            # into separate small tiles so the opening matmuls depend only on
            # tiny DMAs, not on the full-tensor loads running behind them
            nh = min(6, nb0)
            c00 = nh * BLK
            kT_head = persist.tile([BLK, c00], bf16)
            qT_head = persist.tile([BLK, H, c00], bf16)
            v_head = persist.tile([BLK, nh, HEAD_DIM], bf16)
            # one ring, strict order: tiny head tiles land first (the ring
            # serializes transfers), then seq-0 bulk; other seqs + mask go on
            # the Scalar ring where they can't delay the heads
            nc.sync.dma_start(qT_head[:], qT_ext[:, :, :c00])
            nc.sync.dma_start(kT_head[:], kT_ext[:, :c00])
            nc.sync.dma_start(v_head[:], v_re[:, :nh, :])
            nc.sync.dma_start(qT_sb[:, :, : nb0 * BLK],
                              qT_ext[:, :, : nb0 * BLK])
            nc.sync.dma_start(kT_sb[:, : nb0 * BLK], kT_ext[:, : nb0 * BLK])
            nc.sync.dma_start(v_sb[:, :nb0, :], v_re[:, :nb0, :])
            nc.scalar.dma_start(mask_sb[:], mask_ext[:])
            if nb0 < n_blocks_total:
                c0 = nb0 * BLK
                nc.scalar.dma_start(kT_sb[:, c0:], kT_ext[:, c0:])
                nc.scalar.dma_start(v_sb[:, nb0:, :], v_re[:, nb0:, :])
                nc.scalar.dma_start(qT_sb[:, :, c0:], qT_ext[:, :, c0:])

            ones_f = persist.tile([BLK, BLK], f32)
            nc.vector.memset(ones_f[:], 1.0)
            # full [128,128] ones stationary: sums matmul runs M=128 so the
            # PE array never reconfigures col groups between AV and sums
            ones_b = persist.tile([BLK, BLK], bf16)
            nc.vector.tensor_copy(ones_b[:], ones_f[:])

            ot_stage = persist.tile([BLK, H, T], f32)
            sums_stage = persist.tile([1, H, T], f32)

            # flat chunk stream over (seq, q-block g, k-block j-pairs), with
            # j descending inside each group (diagonal/masked block first).
            # Each chunk = up to 2 consecutive j's whose S tiles share one
            # 2-bank PSUM tile [128, jj, h, q] so ONE exp covers both.
            chunks = []
            seq_off = 0
            for nblk in seq_blocks:
                for g in range(nblk):
                    js = list(range(g, -1, -1))
                    for i0 in range(0, len(js), 2):
                        chunks.append((seq_off, nblk, g, js[i0 : i0 + 2]))
                seq_off += nblk * BLK

            # software-pipelined ISSUE order: AV/sums trail their S/exp by
            # LAG chunks so the PE's FIFO never parks on an exp wait
            LAG = 2
            state = {}  # live group accumulators keyed by (seq_off, g)
            pending = []

            def emit_front(ch):
                seq_off, nblk, g, js = ch
                Q0 = seq_off + g * BLK
                early = seq_off == 0 and g < nh
                s2 = ps_s4.tile([BLK, 2, H, BLK], f32, tag="s2", name="s2")
                for jj, j in enumerate(js):
                    if early:
                        kj = kT_head[:, j * BLK : (j + 1) * BLK]
                        qg = qT_head[:, :, g * BLK : (g + 1) * BLK]
                    else:
                        kj = kT_sb[:, seq_off + j * BLK : seq_off + (j + 1) * BLK]
                        qg = qT_sb[:, :, Q0 : Q0 + BLK]
                    nc.tensor.matmul(
                        s2[:, jj], kj, qg,
                        start=True, stop=True,
                    )
                p2 = p_pool.tile([BLK, 2, H, BLK], bf16, tag="p2", name="p2")
                nj = len(js)
                nc.scalar.activation(
                    p2[:, :nj], s2[:, :nj],
                    mybir.ActivationFunctionType.Exp,
                    scale=SCALE,
                )
                if js[0] == g:  # diagonal: zero the upper triangle
                    nc.gpsimd.tensor_mul(p2[:, 0], p2[:, 0], mask_sb[:])
                return p2

            def emit_back(ch, p2):
                seq_off, nblk, g, js = ch
                Q0 = seq_off + g * BLK
                key = (seq_off, g)
                if js[0] == g:
                    state[key] = (
                        ps_o.tile([BLK, H, BLK], f32, tag="ot_ps",
                                  name="oT_ps"),
                        ps_sum.tile([BLK, H, BLK], f32, tag="sums_ps",
                                    name="sums_ps"),
                    )
                oT_ps, sums_ps = state[key]
                for jj, j in enumerate(js):
                    if seq_off == 0 and g < nh:
                        vj = v_head[:, j, :]
                    else:
                        vj = v_sb[:, seq_off // BLK + j, :]
                    nc.tensor.matmul(
                        oT_ps[:], vj, p2[:, jj], start=(j == g), stop=(j == 0)
                    )
                for jj, j in enumerate(js):
                    nc.tensor.matmul(
                        sums_ps[:], ones_b[:], p2[:, jj],
                        start=(j == g), stop=(j == 0),
                    )
                if js[-1] == 0:
                    nc.vector.tensor_copy(
                        ot_stage[:, :, Q0 : Q0 + BLK], oT_ps[:]
                    )
                    nc.vector.tensor_copy(
                        sums_stage[:, :, Q0 : Q0 + BLK], sums_ps[0:1]
                    )
                    del state[key]
                    if g == nblk - 1:  # sequence finished: stream outputs
                        Ls = nblk * BLK
                        if seq_off + Ls == T:
                            nc.sync.dma_start(sums_ext[:], sums_stage[:])
                            for g0 in range(0, Ls, BLK):
                                nc.sync.dma_start(
                                    oT_ext[:, :, seq_off + g0 : seq_off + g0 + BLK],
                                    ot_stage[:, :, seq_off + g0 : seq_off + g0 + BLK],
                                )
                        else:
                            nc.sync.dma_start(
                                oT_ext[:, :, seq_off : seq_off + Ls],
                                ot_stage[:, :, seq_off : seq_off + Ls],
                            )

            for ch in chunks:
                p2 = emit_front(ch)
                pending.append((ch, p2))
                if len(pending) > LAG:
                    emit_back(*pending.pop(0))
            for ch, p2 in pending:
                emit_back(ch, p2)

    nc.finalize()
    return nc


def _install_ntff_hook():
    """Shim antenv.axon_hooks (absent in this container) so trace=True can
    reach the terminal's NRT profiler via libaxon_pjrt.so ctypes."""
    import types

    if "antenv.axon_hooks" in sys.modules:
        return
    import antenv
    from concourse import bass_utils

    mod = types.ModuleType("antenv.axon_hooks")
    state = {"hook": None}
    mod.set_axon_ntff_profile_hook = lambda h: state.__setitem__("hook", h)
    mod.get_axon_ntff_profile_hook = lambda: state["hook"]
    sys.modules["antenv.axon_hooks"] = mod
    antenv.axon_hooks = mod
    bass_utils.upload_artifacts = lambda tmpdir: tmpdir  # zero-egress container
    try:
        if "/root/.axon_site" not in sys.path:
            sys.path.insert(0, "/root/.axon_site")
        from trn_agent_boot.trn_boot import _ntff_profile_via_ctypes

        mod.set_axon_ntff_profile_hook(
            _ntff_profile_via_ctypes("/opt/axon/libaxon_pjrt.so")
        )
    except Exception:
        pass


def kernel(q, k, v, cu_seqlens, max_seqlen):
    from concourse import bass_utils

    q = np.asarray(q, dtype=np.float32)
    k = np.asarray(k, dtype=np.float32)
    v = np.asarray(v, dtype=np.float32)
    cu = np.asarray(cu_seqlens, dtype=np.int64)
    T_host = q.shape[0]
    lengths = np.diff(cu).astype(np.int64)
    all_nblocks = [int((L + BLK - 1) // BLK) for L in lengths]
    T_pad = sum(all_nblocks) * BLK

    # process sequences longest-first: big seq warms the pipeline while the
    # rest of the data streams in, and the tail drains a small seq
    order = sorted(range(len(lengths)), key=lambda s: -all_nblocks[s])
    nblocks = [all_nblocks[s] for s in order]

    # host -> padded device token index map (valid tokens only)
    dev_idx = np.zeros(T_host, dtype=np.int64)
    pad_off = 0
    for s in order:
        L = int(lengths[s])
        dev_idx[cu[s] : cu[s] + L] = pad_off + np.arange(L)
        pad_off += all_nblocks[s] * BLK

    bf16 = ml_dtypes.bfloat16
    qp = np.zeros((T_pad, NUM_HEADS * HEAD_DIM), bf16)
    kp = np.zeros((T_pad, NUM_KV_HEADS * HEAD_DIM), bf16)
    vp = np.zeros((T_pad, NUM_KV_HEADS * HEAD_DIM), bf16)
    qp[dev_idx] = q.astype(bf16)
    kp[dev_idx] = k.astype(bf16)
    vp[dev_idx] = v.astype(bf16)

    mask1 = np.where(
        np.arange(BLK)[:, None] <= np.arange(BLK)[None, :], 1.0, 0.0
    ).astype(bf16)
    mask = np.broadcast_to(
        mask1[:, None, :], (BLK, HEADS_PER_CORE, BLK)
    ).copy()

    key = tuple(nblocks)
    if key not in _GRAPH_CACHE:
        _GRAPH_CACHE[key] = _build_graph(key)
    nc = _GRAPH_CACHE[key]

    in_maps = []
    for c in range(N_CORES):
        m = {"mask": mask}
        m["kT"] = np.ascontiguousarray(kp[:, c * HEAD_DIM : (c + 1) * HEAD_DIM].T)
        m["v"] = np.ascontiguousarray(vp[:, c * HEAD_DIM : (c + 1) * HEAD_DIM])
        # [d, h, t] head-interleaved Q^T so all 4 heads ride one 3D AP
        qc = qp[:, c * HEADS_PER_CORE * HEAD_DIM : (c + 1) * HEADS_PER_CORE * HEAD_DIM]
        m["qT"] = np.ascontiguousarray(
            qc.reshape(T_pad, HEADS_PER_CORE, HEAD_DIM).transpose(2, 1, 0)
        )
        in_maps.append(m)

    trace = bool(os.environ.get("BASS_TRACE"))
    if trace:
        _install_ntff_hook()
    res = bass_utils.run_bass_kernel_spmd(
        nc, in_maps, core_ids=list(range(N_CORES)), trace=trace
    )
    if trace and res.exec_time_ns is not None:
        print(f"HW exec time: {res.exec_time_ns} ns")
        if res.instructions_and_trace is not None:
            print(f"trace: {res.instructions_and_trace[1]}")

    out = np.empty((T_host, NUM_HEADS * HEAD_DIM), np.float32)
    for c in range(N_CORES):
        r = res.results[c]
        oT = r["oT"]  # [128, H, T_pad] unnormalized
        sums = r["sums"][0]  # [H, T_pad]
        for h in range(HEADS_PER_CORE):
            gh = c * HEADS_PER_CORE + h
            o = (oT[:, h][:, dev_idx] / sums[h][dev_idx][None, :]).T
            out[:, gh * HEAD_DIM : (gh + 1) * HEAD_DIM] = o
    return out
